# revision 23
# baseline (speedup 1.0000x reference)
"""Sparse (policy-masked) attention on 8 TRN2 NeuronCores.

Strategy: pure data-parallel over batch (B=8 -> one batch element per core,
weights replicated, no collectives). Each core computes its batch element:

  Prologue: x is PE-transposed (f32r transpose mode, eight tiles batched per
  PSUM group, copies alternating DVE/ScalarE); W_v rows stream in and V is
  computed into a packed [V_h | 1] per-head layout whose ones column makes the
  PV matmul's 65th output row carry the softmax denominator for free.

  Fused main loop: Q^T/K^T projection tiles are JIT-produced inside the head
  loop (weight row DMA -> PE transpose -> qkv matmul on a dedicated PSUM tag)
  between the two heads of each pair, hiding the projection under the
  exp-bound attention window. Per head, S^T = K Q^T in key-major layout makes
  the policy mask a per-partition ACT bias: P^T = Exp(S^T*scale +
  (policy-1)*1024) (exp(-1024) == 0). The always-keep-diagonal rule is
  restored by accumulating (1-policy[m]) * 2^13 onto the raw-score diagonal
  with one tiny bf16 identity matmul per tile; the +1024 - 1024 cancellation
  in the ACT is exact in fp32. The reference's row-max subtraction is skipped
  (scores are O(10); exp cannot overflow) and the EPS/N numerator term
  dropped; both are ~1e-7 relative effects. The reciprocal of (rowsum + EPS)
  is replicated across partitions via a DRAM-bounce broadcast DMA (the
  epilogue stays off the PE), and odd heads' numerators are lane-shifted to
  partitions 64..127 with an identity matmul (out base 64) so the output
  projection runs full K=128 k-tiles.

Measured: ~219 us/core (HW-calibrated cost-model timeline; no NTFF profiling
under this axon client); rel err vs fp64 reference 3.4e-4 on hardware.
PE busy ~167 us, ACT (96 exps) ~111 us, DVE ~103 us.
"""

import numpy as np

import concourse.bass as bass
import concourse.mybir as mybir
import concourse.tile as tile_mod
from concourse.alu_op_type import AluOpType
from concourse.masks import make_identity
from concourse.tile import TileContext


class TC(TileContext):
    """TileContext emitting at most one sync-wait per instruction.

    The pinned walrus rejects any instruction with >1 sem waits
    ("Too many sync wait commands", setupSyncWait), so excess waits are
    hoisted onto single-wait NoOps on the same engine right before the
    instruction, and the final drain is emitted as a drain chain.
    """

    _ww_counter = 0

    def _commit_instruction(self, inst, lazy_reg_writes: bool = True):
        si = getattr(inst, "sync_info", None)
        if si is not None and si.on_wait is not None and len(si.on_wait) > 1:
            waits = list(si.on_wait)
            for w in waits[:-1]:
                TC._ww_counter += 1
                nop = mybir.InstNoOp(
                    name=f"{inst.name}-ww{TC._ww_counter}",
                    engine=inst.engine,
                    sync_info=mybir.SyncInfo(on_wait=[w], on_update=[]),
                    bass_nofuse=True,
                )
                super()._commit_instruction(nop, lazy_reg_writes)
            inst.sync_info = mybir.SyncInfo(
                on_wait=waits[-1:], on_update=list(si.on_update))
        return super()._commit_instruction(inst, lazy_reg_writes)

    def _drain_and_barrier(self, tick_clock, wait_clock):
        drain_inst = self.nc.sync.drain()
        wait_clock.add_sem_waits(
            drain_inst.ins, tile_mod.ScopedClock({None: tick_clock.global_clock})
        )
        waits = list(drain_inst.ins.sync_info.on_wait)
        if len(waits) > 1:
            drain_inst.ins.sync_info = mybir.SyncInfo(on_wait=waits[:1], on_update=[])
            for w in waits[1:]:
                d2 = self.nc.sync.drain()
                d2.ins.sync_info = mybir.SyncInfo(on_wait=[w], on_update=[])
        self.nc.all_engine_barrier()
        assert self.sems is not None
        popped = self.nc._tile_sem_poison_stack.pop()
        assert popped is self._sem_poison
        self.nc.clear_and_free_semaphores(list(self.sems.allocated().values()))
        self.nc.all_engine_barrier()

N, C, H, HD = 1024, 768, 12, 64
B = 8
SCALE = HD ** -0.5
EPS = 1e-6
BIG = 1024.0          # mask bias magnitude (post-scale); exp(-1024) == 0
DVAL = 8192.0         # BIG / SCALE, exactly representable power of two
F32 = mybir.dt.float32
F32R = mybir.dt.float32r
BF16 = mybir.dt.bfloat16
AF = mybir.ActivationFunctionType
NT = N // 128      # 8 n-tiles
CT = C // 128      # 6 c-tiles
QKT = 2 * C // 128  # 12 c_out tiles for Q,K


def build_program():
    nc = bass.Bass()
    x_e = nc.declare_dram_parameter("x", [N, C], F32, isOutput=False)
    pol_e = nc.declare_dram_parameter("policy", [N, 1], F32, isOutput=False)
    wqkv_e = nc.declare_dram_parameter("w_qkv", [3 * C, C], F32, isOutput=False)
    wproj_e = nc.declare_dram_parameter("w_proj", [C, C], F32, isOutput=False)
    b_e = nc.declare_dram_parameter("b_proj", [C], F32, isOutput=False)
    out_e = nc.declare_dram_parameter("out", [N, C], F32, isOutput=True)

    with TC(nc) as tc:
        with tc.tile_pool(name="persist", bufs=1) as pp, \
             tc.tile_pool(name="psA", bufs=2, space="PSUM") as psA, \
             tc.tile_pool(name="psV1", bufs=1, space="PSUM") as psV1, \
             tc.tile_pool(name="psV2", bufs=1, space="PSUM") as psV2, \
             tc.tile_pool(name="psJ", bufs=1, space="PSUM") as psJ:

            # ---- constants ----
            ident = pp.tile([128, 128], F32, tag="ident")
            make_identity(nc, ident[:])
            ident_b = pp.tile([128, 128], BF16, tag="ident_b")
            nc.vector.tensor_copy(ident_b[:], ident[:])
            ident_r = pp.tile([128, 128], F32R, tag="ident_r")
            nc.vector.tensor_copy(ident_r[:], ident[:])
            pol_t = pp.tile([128, NT], F32, tag="pol")
            nc.sync.dma_start(out=pol_t[:], in_=pol_e.rearrange("(t p) o -> p (t o)", p=128))
            # Mask bias: (policy-1)*1024 -> 0 kept, -1024 dropped; exp(-1024)=0.
            logmask = pp.tile([128, NT], F32, tag="logmask")
            nc.vector.tensor_scalar(logmask[:], pol_t[:], -1.0, float(BIG),
                                    AluOpType.add, AluOpType.mult)
            # Diagonal unmask: add (1-policy[m])*2^13 to the raw score diagonal
            # so the ACT bias cancels exactly there (2^13 * SCALE == 1024).
            dpol = pp.tile([128, NT], F32, tag="dpol")
            nc.vector.tensor_scalar(dpol[:], pol_t[:], -1.0, -float(DVAL),
                                    AluOpType.add, AluOpType.mult)
            dmask = [pp.tile([128, 128], BF16, name=f"dmask{t}", tag=f"dmask{t}")
                     for t in range(NT)]
            for t in range(NT):
                nc.vector.tensor_scalar(dmask[t][:], ident[:], dpol[:, t:t + 1],
                                        None, AluOpType.mult)

            ones_f = pp.tile([128, H], F32, tag="ones_f")
            nc.gpsimd.memset(ones_f[:], 1.0)
            ones = pp.tile([128, H], BF16, tag="ones")
            nc.vector.tensor_copy(ones[:], ones_f[:])

            b_bc = pp.tile([128, C], F32, tag="b_bc")
            nc.sync.dma_start(
                out=b_bc[:],
                in_=b_e.rearrange("(o c) -> o c", o=1).to_broadcast([128, C]))

            # ---- persistent products ----
            vaug = [pp.tile([128, H * (HD + 1)], BF16, name=f"vaug{t}", tag=f"vaug{t}") for t in range(NT)]

            def cpy(i, out, in_):
                # alternate copies between DVE and ACT to halve the copy wall
                if i % 2 == 0:
                    nc.vector.tensor_copy(out, in_)
                else:
                    nc.scalar.copy(out, in_)

            def transpose(out, in_):
                nc.tensor.matmul(out, in_, ident_r[:], is_transpose=True,
                                 skip_group_check=True)

            # ---- phase 1: x transposes; V via JIT-transposed W_v ----
            with tc.tile_pool(name="loadL1", bufs=1) as l1p:
                xT = [l1p.tile([128, N], BF16, name=f"xT{c}", tag=f"xT{c}") for c in range(CT)]
                with tc.tile_pool(name="xrawp", bufs=1) as xrawp:
                    xraws = []
                    for nt in range(NT):
                        xr = xrawp.tile([128, C], F32R, name=f"xraw{nt}", tag=f"xraw{nt}")
                        nc.sync.dma_start(out=xr[:], in_=x_e[nt * 128:(nt + 1) * 128, :].bitcast(F32R))
                        xraws.append(xr)
                    for c in range(CT):
                        for half in range(2):
                            psg = psA.tile([128, N // 2], F32R, name="psg", tag="A")
                            for i in range(NT // 2):
                                nt = half * (NT // 2) + i
                                transpose(psg[:, i * 128:(i + 1) * 128],
                                          xraws[nt][:, c * 128:(c + 1) * 128])
                            cpy(c * 2 + half,
                                xT[c][:, half * (N // 2):(half + 1) * (N // 2)], psg[:])
                # pair-0 q/k rows load right behind x; wv after them
                qk0raw = {}
                for t0 in (0, CT):
                    wr0 = l1p.tile([128, C], F32R, name=f"qk0r{t0}", tag=f"qk0r{t0}")
                    nc.sync.dma_start(out=wr0[:], in_=wqkv_e[t0 * 128:(t0 + 1) * 128, :].bitcast(F32R))
                    qk0raw[t0] = wr0
                wvraws = []
                for i in range(CT):
                    rr = 2 * CT + i
                    wr = l1p.tile([128, C], F32R, name=f"wvraw{i}", tag=f"wvraw{i}")
                    nc.sync.dma_start(out=wr[:], in_=wqkv_e[rr * 128:(rr + 1) * 128, :].bitcast(F32R))
                    wvraws.append(wr)
                wvT = [l1p.tile([128, C], BF16, name=f"wvT{c}", tag=f"wvT{c}") for c in range(CT)]
                for nt in range(NT):
                    nc.vector.tensor_copy(
                        vaug[nt][:].rearrange("p (h e) -> p e h", e=HD + 1)[:, HD:HD + 1, :],
                        ones[:, 0:H].rearrange("p (o h) -> p o h", o=1))

                # ---- phase 3: attention with JIT qkv^T between heads ----
                with tc.tile_pool(name="loadL2", bufs=1) as l2p, \
                         tc.tile_pool(name="rawcyc2", bufs=1) as rawp2, \
                         tc.tile_pool(name="jitq", bufs=2) as jitq, \
                         tc.tile_pool(name="qkp", bufs=4) as qkp, \
                         tc.tile_pool(name="work", bufs=2) as wp, \
                         tc.tile_pool(name="ptp", bufs=24) as ptp, \
                         tc.tile_pool(name="epi", bufs=3) as ep:
                    awide = [l2p.tile([128, C], BF16, name=f"aw{t}", tag=f"aw{t}")
                             for t in range(NT)]
                    attnT = l2p.tile([128, CT * N], BF16, tag="attnT")

                    # prefetch w_proj during attention (ACT hwdge queue)
                    wpraws = []
                    for rr in range(CT):
                        wpraw = rawp2.tile([128, C], F32R, name=f"wpraw{rr}", tag=f"wpraw{rr}")
                        nc.scalar.dma_start(out=wpraw[:], in_=wproj_e[rr * 128:(rr + 1) * 128, :].bitcast(F32R))
                        wpraws.append(wpraw)

                    qkT = {}

                    def jit_chunks(t):
                        """Chunked Q^T/K^T projection for w_qkv row-tile t."""
                        if t in qk0raw:
                            wqr = qk0raw[t]
                        else:
                            wqr = jitq.tile([128, C], F32R, name="wqr", tag="wqr")
                            nc.sync.dma_start(out=wqr[:], in_=wqkv_e[t * 128:(t + 1) * 128, :].bitcast(F32R))
                        psg = psJ.tile([128, C], F32R, name="psgq", tag="J")
                        def tchunk():
                            for c in range(CT):
                                transpose(psg[:, c * 128:(c + 1) * 128],
                                          wqr[:, c * 128:(c + 1) * 128])
                        yield tchunk
                        wqTt = jitq.tile([128, C], BF16, name="wqTt", tag="wqTt")
                        yield lambda: nc.vector.tensor_copy(wqTt[:], psg[:])
                        psq = psJ.tile([128, N], F32, name="psJN", tag="J")
                        for c0 in range(0, CT, 2):
                            for j in range(2):
                                def mm(c0=c0, j=j):
                                    for c in (c0, c0 + 1):
                                        nc.tensor.matmul(
                                            psq[:, j * 512:(j + 1) * 512],
                                            wqTt[:, c * 128:(c + 1) * 128],
                                            xT[c][:, j * 512:(j + 1) * 512],
                                            start=(c == 0), stop=(c == CT - 1),
                                            skip_group_check=True)
                                yield mm
                        def fin():
                            qo = qkp.tile([128, N], BF16, name="qkvTt", tag="qk")
                            nc.vector.tensor_copy(qo[:], psq[:])
                            qkT[t] = qo
                        yield fin

                    def wvt_chunks():
                        """W_v^T via PE transposes (chunked)."""
                        for c in range(CT):
                            def tchunk(c=c):
                                psg = psA.tile([128, C], F32R, name="psg2", tag="A")
                                for i in range(CT):
                                    transpose(psg[:, i * 128:(i + 1) * 128],
                                              wvraws[i][:, c * 128:(c + 1) * 128])
                                cpy(c, wvT[c][:], psg[:])
                            yield tchunk

                    def v_chunks(nt):
                        """V projection for n-tile nt into the (idle until the
                        first PV) psV banks; finishes with the bf16 vaug copy."""
                        a = psV1.tile([128, 512], F32, name=f"vA{nt}", tag="V1")
                        bt = psV2.tile([128, 512], F32, name=f"vB{nt}", tag="V2")
                        for c in range(CT):
                            def mm(c=c):
                                nc.tensor.matmul(
                                    a[:, 0:512],
                                    xT[c][:, nt * 128:(nt + 1) * 128],
                                    wvT[c][:, 0:512],
                                    start=(c == 0), stop=(c == CT - 1),
                                    skip_group_check=True)
                                nc.tensor.matmul(
                                    bt[:, 0:256],
                                    xT[c][:, nt * 128:(nt + 1) * 128],
                                    wvT[c][:, 512:768],
                                    start=(c == 0), stop=(c == CT - 1),
                                    skip_group_check=True)
                            yield mm
                        def fin():
                            nc.vector.tensor_copy(
                                vaug[nt][:].rearrange("p (h e) -> p h e", h=H)[:, 0:8, 0:HD],
                                a[:, 0:512].rearrange("p (h e) -> p h e", h=8))
                            nc.vector.tensor_copy(
                                vaug[nt][:].rearrange("p (h e) -> p h e", h=H)[:, 8:12, 0:HD],
                                bt[:, 0:256].rearrange("p (h e) -> p h e", h=4))
                        yield fin

                    filler = []

                    def pop_filler(budget):
                        done = 0
                        while filler and done < budget:
                            try:
                                next(filler[0])()
                                done += 1
                            except StopIteration:
                                filler.pop(0)

                    ptiles = {}   # h -> [8 bf16 P^T tiles]
                    psv = {}      # h -> (psum nt 0-3, psum nt 4-7)

                    def emit_head(h, qt, kt, pv_h):
                        """Scores+exp for head h; PV matmuls for head pv_h
                        (the previously-scored head) interleaved per key-tile;
                        pv_h's normalization at the end."""
                        rb = (h % 2) * 64
                        if pv_h is not None:
                            psv[pv_h] = (
                                psV1.tile([128, 512], F32, name=f"pva{pv_h}", tag="V1"),
                                psV2.tile([128, 512], F32, name=f"pvb{pv_h}", tag="V2"))
                        for mt in range(NT):
                            ps = psA.tile([128, N], F32, name="psN", tag="A")
                            for j in range(2):
                                nc.tensor.matmul(
                                    ps[:, j * 512:(j + 1) * 512],
                                    kt[rb:rb + HD, mt * 128:(mt + 1) * 128],
                                    qt[rb:rb + HD, j * 512:(j + 1) * 512],
                                    start=True, stop=(j != mt // 4),
                                    skip_group_check=True)
                            nc.tensor.matmul(
                                ps[:, mt * 128:(mt + 1) * 128],
                                ident_b[:], dmask[mt][:],
                                start=False, stop=True, skip_group_check=True)
                            ptile = ptp.tile([128, N], BF16, name=f"pt{h}_{mt}", tag="pt")
                            nc.scalar.activation(ptile[:], ps[:], AF.Exp,
                                                 bias=logmask[:, mt:mt + 1], scale=SCALE)
                            ptiles.setdefault(h, []).append(ptile)
                            if pv_h is not None:
                                emit_pv_nt(pv_h, mt)
                            pop_filler(2 if pv_h is not None else 8)
                        if pv_h is not None:
                            emit_norm(pv_h)

                    def emit_pv_nt(h, nt):
                        # query-major PV for one query tile: stationary P^T
                        # block [128 keys, 128 queries], moving [V | 1]
                        # [128 keys, 65] -> psum [128 queries, 65] with the
                        # denominator in col 64. The 8 key-tile matmuls are
                        # contiguous: one PSUM bank allows only one open
                        # accumulation group at a time.
                        a, bt = psv[h]
                        tgt = a if nt < 4 else bt
                        off = (nt % 4) * 128
                        for mt in range(NT):
                            nc.tensor.matmul(
                                tgt[:, off:off + HD + 1],
                                ptiles[h][mt][:, nt * 128:(nt + 1) * 128],
                                vaug[mt][:, h * (HD + 1):(h + 1) * (HD + 1)],
                                start=(mt == 0), stop=(mt == NT - 1),
                                skip_group_check=True)

                    def emit_norm(h):
                        a, bt = psv[h]
                        den = ep.tile([128, NT], F32, name=f"den{h}", tag="den")
                        for i, t in enumerate((a, bt)):
                            dv = t[:].rearrange("p (nt e) -> p nt e", e=128)[:, :, HD:HD + 1]
                            nc.vector.tensor_scalar_add(den[:, i * 4:(i + 1) * 4], dv, EPS)
                        nc.vector.reciprocal(den[:], den[:])
                        for nt in range(NT):
                            t = a if nt < 4 else bt
                            off = (nt % 4) * 128
                            nc.vector.tensor_scalar(
                                awide[nt][:, h * HD:(h + 1) * HD],
                                t[:, off:off + HD], den[:, nt:nt + 1], None,
                                AluOpType.mult)
                        del psv[h]
                        del ptiles[h]

                    # pair-0 q/k projected up-front (before V, right after xT)
                    filler.append(jit_chunks(0))
                    filler.append(jit_chunks(CT))
                    pop_filler(100)
                    # wvT + all of V run as filler inside the first two score
                    # blocks (heads 1 and 0); PV lags its head by two blocks.
                    filler.append(wvt_chunks())
                    for nt in range(NT):
                        filler.append(v_chunks(nt))

                    seq = []
                    for tp in range(CT):
                        seq += [2 * tp + 1, 2 * tp]
                    for bi, h in enumerate(seq):
                        tp = h // 2
                        if h % 2 == 1 and tp + 1 < CT:
                            filler.append(jit_chunks(tp + 1))
                            filler.append(jit_chunks(CT + tp + 1))
                        if h % 2 == 1 and (tp not in qkT or CT + tp not in qkT):
                            # this pair's q/k must be fully emitted first
                            while tp not in qkT or CT + tp not in qkT:
                                pop_filler(1)
                        pvs = [None, None, 1, 0, 3, 2, 5, 4, 7, 6, 9, 11]
                        emit_head(h, qkT[tp], qkT[CT + tp], pvs[bi])

                    # ---- phase 1b: w_proj transposes (fill the attention tail) ----
                    wpT = [l2p.tile([128, C], BF16, name=f"wpT{p}", tag=f"wpT{p}")
                           for p in range(H // 2)]
                    for hp in range(H // 2):
                        psg = psJ.tile([128, C], F32R, name="psg3", tag="J")
                        for rr in range(CT):
                            transpose(psg[:, rr * 128:(rr + 1) * 128],
                                      wpraws[rr][:, hp * 128:(hp + 1) * 128])
                        cpy(hp, wpT[hp][:], psg[:])

                    # trailing PVs: h8 (ready since block 9) then h10
                    for th in (8, 10):
                        psv[th] = (
                            psV1.tile([128, 512], F32, name=f"pva{th}", tag="V1"),
                            psV2.tile([128, 512], F32, name=f"pvb{th}", tag="V2"))
                        for nt in range(NT):
                            emit_pv_nt(th, nt)
                        emit_norm(th)

                    # attn-out -> c-major PE transposes pipelined with the
                    # projection: transpose(nt+1) hides proj(nt)'s wait on the
                    # attnT copy. (block (c, nt) of attnT at col c*N + nt*128)
                    def emit_trans(nt):
                        psg = psJ.tile([128, C], BF16, name="psgA", tag="J")
                        for c in range(CT):
                            nc.tensor.matmul(psg[:, c * 128:(c + 1) * 128],
                                             awide[nt][:, c * 128:(c + 1) * 128],
                                             ident_b[:], is_transpose=True,
                                             skip_group_check=True)
                        cpy(nt, attnT[:].rearrange("p (t nn) -> p t nn", t=CT)[
                            :, :, nt * 128:nt * 128 + 128], psg[:].rearrange(
                            "p (t nn) -> p t nn", t=CT))

                    def emit_proj(nt):
                        ps = psA.tile([128, C], F32, name="psC2", tag="A")
                        for hp in range(H // 2):
                            for f0, fw in ((0, 512), (512, 256)):
                                nc.tensor.matmul(
                                    ps[:, f0:f0 + fw],
                                    attnT[:, hp * N + nt * 128: hp * N + nt * 128 + 128],
                                    wpT[hp][:, f0:f0 + fw],
                                    start=(hp == 0), stop=(hp == H // 2 - 1))
                        yt = wp.tile([128, C], F32, name="yt", tag="yt")
                        nc.vector.tensor_tensor(yt[:], ps[:], b_bc[:], AluOpType.add)
                        nc.sync.dma_start(out=out_e[nt * 128:(nt + 1) * 128, :], in_=yt[:])

                    emit_trans(0)
                    for nt in range(1, NT):
                        emit_trans(nt)
                        emit_proj(nt - 1)
                    emit_proj(NT - 1)

    return nc


_NC = None


def _get_nc():
    global _NC
    if _NC is None:
        _NC = build_program()
    return _NC


def run(in_maps, trace=False, **kw):
    from concourse.bass_utils import run_bass_kernel_spmd
    return run_bass_kernel_spmd(_get_nc(), in_maps, core_ids=list(range(B)),
                                trace=trace, **kw)


def kernel(x, policy, w_qkv, w_proj, b_proj):
    x = np.ascontiguousarray(np.asarray(x, dtype=np.float32))
    policy = np.ascontiguousarray(np.asarray(policy, dtype=np.float32))
    w_qkv = np.ascontiguousarray(np.asarray(w_qkv, dtype=np.float32))
    w_proj = np.ascontiguousarray(np.asarray(w_proj, dtype=np.float32))
    b_proj = np.ascontiguousarray(np.asarray(b_proj, dtype=np.float32))
    in_maps = [
        {"x": x[i], "policy": policy[i], "w_qkv": w_qkv,
         "w_proj": w_proj, "b_proj": b_proj}
        for i in range(B)
    ]
    try:
        res = run(in_maps)
    except Exception:
        # one observed transient NRT_EXEC_UNIT_UNRECOVERABLE wedge in ~40
        # invocations this session; a retry is free insurance
        res = run(in_maps)
    return np.stack([res.results[i]["out"] for i in range(B)], axis=0)


if __name__ == "__main__":
    rng = np.random.default_rng(0)
    x = rng.standard_normal((B, N, C), dtype=np.float32)
    policy = (rng.random((B, N, 1)) > 0.3).astype(np.float32)
    w_qkv = rng.standard_normal((3 * C, C), dtype=np.float32) * C ** -0.5
    w_proj = rng.standard_normal((C, C), dtype=np.float32) * C ** -0.5
    b_proj = np.zeros((C,), dtype=np.float32)
    y = kernel(x=x, policy=policy, w_qkv=w_qkv, w_proj=w_proj, b_proj=b_proj)
    print("out", y.shape, y.dtype, np.abs(y).mean())



# revision 31
# speedup vs baseline: 1.0465x; 1.0465x over previous
"""Sparse (policy-masked) attention on 8 TRN2 NeuronCores.

Data-parallel over batch (B=8 -> one batch element per core, weights
replicated, no collectives). Per core:

  Prologue: x and the pair-0 q/k weight rows load first; x is PE-transposed
  (f32r), pair-0 Q^T/K^T are projected immediately (q on the JIT psum bank,
  k on the score bank) so the exp stream starts ~20us in. W_v transposes and
  the V projection run as filler chunks inside the first two score blocks,
  with V accumulating in the then-idle PV psum banks.

  Attention: heads are processed in pair order [1,0,3,2,...,10,11]. Per
  key-tile iteration: S^T = K Q^T (bf16, key-major so the policy mask is a
  per-partition ACT bias), a bf16 identity matmul restores the always-keep
  diagonal (+2^13 pre-scale cancels the -1024 mask bias exactly), then
  exp -> bf16 P^T tiles. PV for a two-block-lagged head interleaves per
  iteration in query-major form: out[128 queries, 65] = P^T-block
  (stationary) @ [V | 1] (moving) -- all 128 output partitions used, the
  denominator rides along as column 64, and each query-tile normalizes on
  DVE (reciprocal + per-partition scale) the moment its column finishes.
  The next pair's Q^T/K^T projection and, late in the run, the W_proj
  transposes are drip-fed between iterations as filler chunks.

  Tail: PV for heads 8 and 11, then attn-out is PE-transposed (bf16) to
  c-major and the output projection (bf16 x bf16) pipelines per n-tile
  with the transposes; bias add on DVE, stores on the sync queue.

Timing source: HW-calibrated cost-model timeline sim (no NTFF profiling
under this axon client). ~185.5 us/core vs 216.7 us for the v1 baseline;
rel err vs fp64 reference 5.5e-3 on hardware (bf16 operand rounding).
"""

import numpy as np

import concourse.bass as bass
import concourse.mybir as mybir
import concourse.tile as tile_mod
from concourse.alu_op_type import AluOpType
from concourse.masks import make_identity
from concourse.tile import TileContext


class TC(TileContext):
    """TileContext emitting at most one sync-wait per instruction.

    The pinned walrus rejects any instruction with >1 sem waits
    ("Too many sync wait commands", setupSyncWait), so excess waits are
    hoisted onto single-wait NoOps on the same engine right before the
    instruction, and the final drain is emitted as a drain chain.
    """

    _ww_counter = 0

    def _commit_instruction(self, inst, lazy_reg_writes: bool = True):
        si = getattr(inst, "sync_info", None)
        if si is not None and si.on_wait is not None and len(si.on_wait) > 1:
            waits = list(si.on_wait)
            for w in waits[:-1]:
                TC._ww_counter += 1
                nop = mybir.InstNoOp(
                    name=f"{inst.name}-ww{TC._ww_counter}",
                    engine=inst.engine,
                    sync_info=mybir.SyncInfo(on_wait=[w], on_update=[]),
                    bass_nofuse=True,
                )
                super()._commit_instruction(nop, lazy_reg_writes)
            inst.sync_info = mybir.SyncInfo(
                on_wait=waits[-1:], on_update=list(si.on_update))
        return super()._commit_instruction(inst, lazy_reg_writes)

    def _drain_and_barrier(self, tick_clock, wait_clock):
        drain_inst = self.nc.sync.drain()
        wait_clock.add_sem_waits(
            drain_inst.ins, tile_mod.ScopedClock({None: tick_clock.global_clock})
        )
        waits = list(drain_inst.ins.sync_info.on_wait)
        if len(waits) > 1:
            drain_inst.ins.sync_info = mybir.SyncInfo(on_wait=waits[:1], on_update=[])
            for w in waits[1:]:
                d2 = self.nc.sync.drain()
                d2.ins.sync_info = mybir.SyncInfo(on_wait=[w], on_update=[])
        self.nc.all_engine_barrier()
        assert self.sems is not None
        popped = self.nc._tile_sem_poison_stack.pop()
        assert popped is self._sem_poison
        self.nc.clear_and_free_semaphores(list(self.sems.allocated().values()))
        self.nc.all_engine_barrier()

N, C, H, HD = 1024, 768, 12, 64
B = 8
SCALE = HD ** -0.5
EPS = 1e-6
BIG = 1024.0          # mask bias magnitude (post-scale); exp(-1024) == 0
DVAL = 8192.0         # BIG / SCALE, exactly representable power of two
F32 = mybir.dt.float32
F32R = mybir.dt.float32r
BF16 = mybir.dt.bfloat16
AF = mybir.ActivationFunctionType
NT = N // 128      # 8 n-tiles
CT = C // 128      # 6 c-tiles
QKT = 2 * C // 128  # 12 c_out tiles for Q,K


def build_program():
    nc = bass.Bass()
    x_e = nc.declare_dram_parameter("x", [N, C], F32, isOutput=False)
    pol_e = nc.declare_dram_parameter("policy", [N, 1], F32, isOutput=False)
    wqkv_e = nc.declare_dram_parameter("w_qkv", [3 * C, C], F32, isOutput=False)
    wproj_e = nc.declare_dram_parameter("w_proj", [C, C], F32, isOutput=False)
    b_e = nc.declare_dram_parameter("b_proj", [C], F32, isOutput=False)
    out_e = nc.declare_dram_parameter("out", [N, C], F32, isOutput=True)

    with TC(nc) as tc:
        with tc.tile_pool(name="persist", bufs=1) as pp, \
             tc.tile_pool(name="psA", bufs=2, space="PSUM") as psA, \
             tc.tile_pool(name="psV1", bufs=1, space="PSUM") as psV1, \
             tc.tile_pool(name="psV2", bufs=1, space="PSUM") as psV2, \
             tc.tile_pool(name="psJ", bufs=1, space="PSUM") as psJ:

            # ---- constants ----
            ident = pp.tile([128, 128], F32, tag="ident")
            make_identity(nc, ident[:])
            ident_b = pp.tile([128, 128], BF16, tag="ident_b")
            nc.vector.tensor_copy(ident_b[:], ident[:])
            ident_r = pp.tile([128, 128], F32R, tag="ident_r")
            nc.vector.tensor_copy(ident_r[:], ident[:])
            pol_t = pp.tile([128, NT], F32, tag="pol")
            nc.sync.dma_start(out=pol_t[:], in_=pol_e.rearrange("(t p) o -> p (t o)", p=128))
            # Mask bias: (policy-1)*1024 -> 0 kept, -1024 dropped; exp(-1024)=0.
            logmask = pp.tile([128, NT], F32, tag="logmask")
            nc.vector.tensor_scalar(logmask[:], pol_t[:], -1.0, float(BIG),
                                    AluOpType.add, AluOpType.mult)
            # Diagonal unmask: add (1-policy[m])*2^13 to the raw score diagonal
            # so the ACT bias cancels exactly there (2^13 * SCALE == 1024).
            dpol = pp.tile([128, NT], F32, tag="dpol")
            nc.vector.tensor_scalar(dpol[:], pol_t[:], -1.0, -float(DVAL),
                                    AluOpType.add, AluOpType.mult)
            dmask = [pp.tile([128, 128], BF16, name=f"dmask{t}", tag=f"dmask{t}")
                     for t in range(NT)]
            for t in range(NT):
                nc.vector.tensor_scalar(dmask[t][:], ident[:], dpol[:, t:t + 1],
                                        None, AluOpType.mult)

            ones_f = pp.tile([128, H], F32, tag="ones_f")
            nc.gpsimd.memset(ones_f[:], 1.0)
            ones = pp.tile([128, H], BF16, tag="ones")
            nc.vector.tensor_copy(ones[:], ones_f[:])

            b_bc = pp.tile([128, C], F32, tag="b_bc")
            nc.sync.dma_start(
                out=b_bc[:],
                in_=b_e.rearrange("(o c) -> o c", o=1).to_broadcast([128, C]))

            # ---- persistent products ----
            vaug = [pp.tile([128, H * (HD + 1)], BF16, name=f"vaug{t}", tag=f"vaug{t}") for t in range(NT)]

            def cpy(i, out, in_):
                # alternate copies between DVE and ACT to halve the copy wall
                if i % 2 == 0:
                    nc.vector.tensor_copy(out, in_)
                else:
                    nc.scalar.copy(out, in_)

            def transpose(out, in_):
                nc.tensor.matmul(out, in_, ident_r[:], is_transpose=True,
                                 skip_group_check=True)

            # ---- phase 1: x transposes; V via JIT-transposed W_v ----
            with tc.tile_pool(name="loadL1", bufs=1) as l1p:
                xT = [l1p.tile([128, N], BF16, name=f"xT{c}", tag=f"xT{c}") for c in range(CT)]
                with tc.tile_pool(name="xrawp", bufs=1) as xrawp:
                    xraws = []
                    for nt in range(NT):
                        xr = xrawp.tile([128, C], F32R, name=f"xraw{nt}", tag=f"xraw{nt}")
                        nc.sync.dma_start(out=xr[:], in_=x_e[nt * 128:(nt + 1) * 128, :].bitcast(F32R))
                        xraws.append(xr)
                    for c in range(CT):
                        for half in range(2):
                            psg = psA.tile([128, N // 2], F32R, name="psg", tag="A")
                            for i in range(NT // 2):
                                nt = half * (NT // 2) + i
                                transpose(psg[:, i * 128:(i + 1) * 128],
                                          xraws[nt][:, c * 128:(c + 1) * 128])
                            cpy(c * 2 + half,
                                xT[c][:, half * (N // 2):(half + 1) * (N // 2)], psg[:])
                # pair-0 q/k rows load right behind x; wv after them
                qk0raw = {}
                for t0 in (0, CT):
                    wr0 = l1p.tile([128, C], F32R, name=f"qk0r{t0}", tag=f"qk0r{t0}")
                    nc.sync.dma_start(out=wr0[:], in_=wqkv_e[t0 * 128:(t0 + 1) * 128, :].bitcast(F32R))
                    qk0raw[t0] = wr0
                wvraws = []
                for i in range(CT):
                    rr = 2 * CT + i
                    wr = l1p.tile([128, C], F32R, name=f"wvraw{i}", tag=f"wvraw{i}")
                    nc.sync.dma_start(out=wr[:], in_=wqkv_e[rr * 128:(rr + 1) * 128, :].bitcast(F32R))
                    wvraws.append(wr)
                wvT = [l1p.tile([128, C], BF16, name=f"wvT{c}", tag=f"wvT{c}") for c in range(CT)]
                for nt in range(NT):
                    nc.vector.tensor_copy(
                        vaug[nt][:].rearrange("p (h e) -> p e h", e=HD + 1)[:, HD:HD + 1, :],
                        ones[:, 0:H].rearrange("p (o h) -> p o h", o=1))

                # ---- phase 3: attention with JIT qkv^T between heads ----
                with tc.tile_pool(name="loadL2", bufs=1) as l2p, \
                         tc.tile_pool(name="rawcyc2", bufs=1) as rawp2, \
                         tc.tile_pool(name="jitq", bufs=2) as jitq, \
                         tc.tile_pool(name="qkp", bufs=4) as qkp, \
                         tc.tile_pool(name="work", bufs=2) as wp, \
                         tc.tile_pool(name="ptp", bufs=24) as ptp, \
                         tc.tile_pool(name="epi", bufs=3) as ep:
                    awide = [l2p.tile([128, C], BF16, name=f"aw{t}", tag=f"aw{t}")
                             for t in range(NT)]
                    attnT = l2p.tile([128, CT * N], BF16, tag="attnT")

                    wpraws = []
                    wpT = [l2p.tile([128, C], BF16, name=f"wpT{p}", tag=f"wpT{p}")
                           for p in range(H // 2)]

                    def wpt_chunks():
                        for hp in range(H // 2):
                            def tchunk(hp=hp):
                                psg = psJ.tile([128, C], F32R, name="psg3", tag="J")
                                for rr in range(CT):
                                    transpose(psg[:, rr * 128:(rr + 1) * 128],
                                              wpraws[rr][:, hp * 128:(hp + 1) * 128])
                                cpy(hp, wpT[hp][:], psg[:])
                            yield tchunk

                    def load_wproj():
                        # deferred prefetch: issued mid-attention so it never
                        # competes with the x/w_qkv loads for DMA bandwidth
                        for rr in range(CT):
                            wpraw = rawp2.tile([128, C], F32R, name=f"wpraw{rr}", tag=f"wpraw{rr}")
                            nc.sync.dma_start(out=wpraw[:], in_=wproj_e[rr * 128:(rr + 1) * 128, :].bitcast(F32R))
                            wpraws.append(wpraw)

                    qkT = {}

                    def jit_chunks(t, pool=None):
                        """Chunked Q^T/K^T projection for w_qkv row-tile t."""
                        pool = pool or psJ
                        tag = "J" if pool is psJ else "A"
                        if t in qk0raw:
                            wqr = qk0raw[t]
                        else:
                            wqr = jitq.tile([128, C], F32R, name="wqr", tag="wqr")
                            nc.sync.dma_start(out=wqr[:], in_=wqkv_e[t * 128:(t + 1) * 128, :].bitcast(F32R))
                        psg = pool.tile([128, C], F32R, name="psgq", tag=tag)
                        def tchunk():
                            for c in range(CT):
                                transpose(psg[:, c * 128:(c + 1) * 128],
                                          wqr[:, c * 128:(c + 1) * 128])
                        yield tchunk
                        wqTt = jitq.tile([128, C], BF16, name="wqTt", tag="wqTt")
                        yield lambda: nc.vector.tensor_copy(wqTt[:], psg[:])
                        psq = pool.tile([128, N], F32, name="psJN", tag=tag)
                        for c0 in range(0, CT, 2):
                            for j in range(2):
                                def mm(c0=c0, j=j):
                                    for c in (c0, c0 + 1):
                                        nc.tensor.matmul(
                                            psq[:, j * 512:(j + 1) * 512],
                                            wqTt[:, c * 128:(c + 1) * 128],
                                            xT[c][:, j * 512:(j + 1) * 512],
                                            start=(c == 0), stop=(c == CT - 1),
                                            skip_group_check=True)
                                yield mm
                        def fin():
                            qo = qkp.tile([128, N], BF16, name="qkvTt", tag="qk")
                            nc.vector.tensor_copy(qo[:], psq[:])
                            qkT[t] = qo
                        yield fin

                    def wvt_chunks():
                        """W_v^T via PE transposes (chunked)."""
                        for c in range(CT):
                            def tchunk(c=c):
                                psg = psA.tile([128, C], F32R, name="psg2", tag="A")
                                for i in range(CT):
                                    transpose(psg[:, i * 128:(i + 1) * 128],
                                              wvraws[i][:, c * 128:(c + 1) * 128])
                                cpy(c, wvT[c][:], psg[:])
                            yield tchunk

                    def v_chunks(nt):
                        """V projection for n-tile nt into the (idle until the
                        first PV) psV banks; finishes with the bf16 vaug copy."""
                        a = psV1.tile([128, 512], F32, name=f"vA{nt}", tag="V1")
                        bt = psV2.tile([128, 512], F32, name=f"vB{nt}", tag="V2")
                        for c in range(CT):
                            def mm(c=c):
                                nc.tensor.matmul(
                                    a[:, 0:512],
                                    xT[c][:, nt * 128:(nt + 1) * 128],
                                    wvT[c][:, 0:512],
                                    start=(c == 0), stop=(c == CT - 1),
                                    skip_group_check=True)
                                nc.tensor.matmul(
                                    bt[:, 0:256],
                                    xT[c][:, nt * 128:(nt + 1) * 128],
                                    wvT[c][:, 512:768],
                                    start=(c == 0), stop=(c == CT - 1),
                                    skip_group_check=True)
                            yield mm
                        def fin():
                            nc.vector.tensor_copy(
                                vaug[nt][:].rearrange("p (h e) -> p h e", h=H)[:, 0:8, 0:HD],
                                a[:, 0:512].rearrange("p (h e) -> p h e", h=8))
                            nc.vector.tensor_copy(
                                vaug[nt][:].rearrange("p (h e) -> p h e", h=H)[:, 8:12, 0:HD],
                                bt[:, 0:256].rearrange("p (h e) -> p h e", h=4))
                        yield fin

                    filler = []

                    def pop_filler(budget):
                        done = 0
                        while filler and done < budget:
                            try:
                                next(filler[0])()
                                done += 1
                            except StopIteration:
                                filler.pop(0)

                    ptiles = {}   # h -> [8 bf16 P^T tiles]
                    psv = {}      # h -> (psum nt 0-3, psum nt 4-7)

                    def psv_alloc(h, pool="V"):
                        if pool == "V":
                            psv[h] = (
                                psV1.tile([128, 512], F32, name=f"pva{h}", tag="V1"),
                                psV2.tile([128, 512], F32, name=f"pvb{h}", tag="V2"))
                        else:
                            # the JIT psum bank is free late in the run; host a
                            # second concurrent PV stream there
                            tile = psJ.tile([128, N], F32, name=f"pvj{h}", tag="J")
                            psv[h] = (tile[:, 0:512], tile[:, 512:1024])

                    def emit_head(h, qt, kt, pv_hs):
                        """Scores+exp for head h; PV matmuls for the lagged
                        heads in pv_hs interleaved per key-tile, each
                        normalized per query-tile as its column finishes."""
                        rb = (h % 2) * 64
                        for i, pv_h in enumerate(pv_hs):
                            psv_alloc(pv_h, "V" if i == 0 else "J")
                        for mt in range(NT):
                            ps = psA.tile([128, N], F32, name="psN", tag="A")
                            for j in range(2):
                                nc.tensor.matmul(
                                    ps[:, j * 512:(j + 1) * 512],
                                    kt[rb:rb + HD, mt * 128:(mt + 1) * 128],
                                    qt[rb:rb + HD, j * 512:(j + 1) * 512],
                                    start=True, stop=(j != mt // 4),
                                    skip_group_check=True)
                            nc.tensor.matmul(
                                ps[:, mt * 128:(mt + 1) * 128],
                                ident_b[:], dmask[mt][:],
                                start=False, stop=True, skip_group_check=True)
                            ptile = ptp.tile([128, N], BF16, name=f"pt{h}_{mt}", tag="pt")
                            nc.scalar.activation(ptile[:], ps[:], AF.Exp,
                                                 bias=logmask[:, mt:mt + 1], scale=SCALE)
                            ptiles.setdefault(h, []).append(ptile)
                            for pv_h in pv_hs:
                                emit_pv_nt(pv_h, mt)
                                emit_norm_nt(pv_h, mt)
                            pop_filler(2 if pv_hs else 8)
                        for pv_h in pv_hs:
                            del psv[pv_h]
                            del ptiles[pv_h]

                    def emit_pv_nt(h, nt):
                        # query-major PV for one query tile: stationary P^T
                        # block [128 keys, 128 queries], moving [V | 1]
                        # [128 keys, 65] -> psum [128 queries, 65] with the
                        # denominator in col 64. The 8 key-tile matmuls are
                        # contiguous: one PSUM bank allows only one open
                        # accumulation group at a time.
                        a, bt = psv[h]
                        tgt = a if nt < 4 else bt
                        off = (nt % 4) * 128
                        for mt in range(NT):
                            nc.tensor.matmul(
                                tgt[:, off:off + HD + 1],
                                ptiles[h][mt][:, nt * 128:(nt + 1) * 128],
                                vaug[mt][:, h * (HD + 1):(h + 1) * (HD + 1)],
                                start=(mt == 0), stop=(mt == NT - 1),
                                skip_group_check=True)

                    dens = {}

                    def emit_norm_nt(h, nt):
                        # normalize query-tile nt of head h right after its PV
                        # column finishes, so the next block's PV stream never
                        # waits on a batched norm chain
                        a, bt = psv[h]
                        t = a if nt < 4 else bt
                        off = (nt % 4) * 128
                        if nt == 0:
                            dens[h] = ep.tile([128, NT], F32, name=f"den{h}", tag="den")
                        den = dens[h]
                        nc.vector.tensor_scalar_add(
                            den[:, nt:nt + 1], t[:, off + HD:off + HD + 1], EPS)
                        nc.vector.reciprocal(den[:, nt:nt + 1], den[:, nt:nt + 1])
                        nc.vector.tensor_scalar(
                            awide[nt][:, h * HD:(h + 1) * HD],
                            t[:, off:off + HD], den[:, nt:nt + 1], None,
                            AluOpType.mult)

                    # pair-0 q/k projected up-front (before V, right after xT)
                    filler.append(jit_chunks(0))
                    filler.append(jit_chunks(CT))
                    pop_filler(100)
                    # wvT + all of V run as filler inside the first two score
                    # blocks (heads 1 and 0); PV lags its head by two blocks.
                    filler.append(wvt_chunks())
                    for nt in range(NT):
                        filler.append(v_chunks(nt))

                    seq = []
                    for tp in range(CT):
                        seq += [2 * tp + 1, 2 * tp]
                    seq[10], seq[11] = 10, 11   # last pair even-first: PV(10)
                    # then rides block 11 and only PV(11) trails the loop
                    for bi, h in enumerate(seq):
                        tp = h // 2
                        if h % 2 == 1 and tp + 1 < CT:
                            filler.append(jit_chunks(tp + 1))
                            filler.append(jit_chunks(CT + tp + 1))
                        if h % 2 == 1 and (tp not in qkT or CT + tp not in qkT):
                            # this pair's q/k must be fully emitted first
                            while tp not in qkT or CT + tp not in qkT:
                                pop_filler(1)
                        if bi == 6:
                            load_wproj()
                        if bi == 8:
                            filler.append(wpt_chunks())
                        pvs = [(), (), (1,), (0,), (3,), (2,), (5,), (4,),
                               (7,), (6,), (9,), (10,)]
                        emit_head(h, qkT[tp], qkT[CT + tp], pvs[bi])

                    # trailing PVs: h8 (ready immediately, on the psJ bank)
                    # and h11 (gated on the final block's exps)
                    psv_alloc(8, "J")
                    for nt in range(NT):
                        emit_pv_nt(8, nt)
                        emit_norm_nt(8, nt)
                    del psv[8]
                    del ptiles[8]
                    psv_alloc(11, "V")
                    for nt in range(NT):
                        emit_pv_nt(11, nt)
                        emit_norm_nt(11, nt)
                    del psv[11]
                    del ptiles[11]

                    # attn-out -> c-major PE transposes pipelined with the
                    # projection: transpose(nt+1) hides proj(nt)'s wait on the
                    # attnT copy. (block (c, nt) of attnT at col c*N + nt*128)
                    def emit_trans(nt):
                        psg = psJ.tile([128, C], BF16, name="psgA", tag="J")
                        for c in range(CT):
                            nc.tensor.matmul(psg[:, c * 128:(c + 1) * 128],
                                             awide[nt][:, c * 128:(c + 1) * 128],
                                             ident_b[:], is_transpose=True,
                                             skip_group_check=True)
                        cpy(nt, attnT[:].rearrange("p (t nn) -> p t nn", t=CT)[
                            :, :, nt * 128:nt * 128 + 128], psg[:].rearrange(
                            "p (t nn) -> p t nn", t=CT))

                    def emit_proj(nt):
                        ps = psA.tile([128, C], F32, name="psC2", tag="A")
                        for hp in range(H // 2):
                            for f0, fw in ((0, 512), (512, 256)):
                                nc.tensor.matmul(
                                    ps[:, f0:f0 + fw],
                                    attnT[:, hp * N + nt * 128: hp * N + nt * 128 + 128],
                                    wpT[hp][:, f0:f0 + fw],
                                    start=(hp == 0), stop=(hp == H // 2 - 1))
                        yt = wp.tile([128, C], F32, name="yt", tag="yt")
                        nc.vector.tensor_tensor(yt[:], ps[:], b_bc[:], AluOpType.add)
                        nc.sync.dma_start(out=out_e[nt * 128:(nt + 1) * 128, :], in_=yt[:])

                    emit_trans(0)
                    for nt in range(1, NT):
                        emit_trans(nt)
                        emit_proj(nt - 1)
                    emit_proj(NT - 1)

    return nc


_NC = None


def _get_nc():
    global _NC
    if _NC is None:
        _NC = build_program()
    return _NC


def run(in_maps, trace=False, **kw):
    from concourse.bass_utils import run_bass_kernel_spmd
    return run_bass_kernel_spmd(_get_nc(), in_maps, core_ids=list(range(B)),
                                trace=trace, **kw)


def kernel(x, policy, w_qkv, w_proj, b_proj):
    x = np.ascontiguousarray(np.asarray(x, dtype=np.float32))
    policy = np.ascontiguousarray(np.asarray(policy, dtype=np.float32))
    w_qkv = np.ascontiguousarray(np.asarray(w_qkv, dtype=np.float32))
    w_proj = np.ascontiguousarray(np.asarray(w_proj, dtype=np.float32))
    b_proj = np.ascontiguousarray(np.asarray(b_proj, dtype=np.float32))
    in_maps = [
        {"x": x[i], "policy": policy[i], "w_qkv": w_qkv,
         "w_proj": w_proj, "b_proj": b_proj}
        for i in range(B)
    ]
    try:
        res = run(in_maps)
    except Exception:
        # one observed transient NRT_EXEC_UNIT_UNRECOVERABLE wedge in ~40
        # invocations this session; a retry is free insurance
        res = run(in_maps)
    return np.stack([res.results[i]["out"] for i in range(B)], axis=0)


if __name__ == "__main__":
    rng = np.random.default_rng(0)
    x = rng.standard_normal((B, N, C), dtype=np.float32)
    policy = (rng.random((B, N, 1)) > 0.3).astype(np.float32)
    w_qkv = rng.standard_normal((3 * C, C), dtype=np.float32) * C ** -0.5
    w_proj = rng.standard_normal((C, C), dtype=np.float32) * C ** -0.5
    b_proj = np.zeros((C,), dtype=np.float32)
    y = kernel(x=x, policy=policy, w_qkv=w_qkv, w_proj=w_proj, b_proj=b_proj)
    print("out", y.shape, y.dtype, np.abs(y).mean())



# revision 40
# speedup vs baseline: 1.0653x; 1.0180x over previous
"""Sparse (policy-masked) attention on 8 TRN2 NeuronCores.

Data-parallel over batch (B=8 -> one batch element per core, weights
replicated, no collectives). Per core:

  Prologue: x and the pair-0 q/k weight rows load first; x is PE-transposed
  (f32r), pair-0 Q^T/K^T are projected immediately (q on the JIT psum bank,
  k on the score bank) so the exp stream starts ~20us in. W_v transposes and
  the V projection run as filler chunks inside the first two score blocks,
  with V accumulating in the then-idle PV psum banks.

  Attention: heads are processed in pair order [1,0,3,2,...,10,11]. Per
  key-tile iteration: S^T = K Q^T (bf16, key-major so the policy mask is a
  per-partition ACT bias), a bf16 identity matmul restores the always-keep
  diagonal (+2^13 pre-scale cancels the -1024 mask bias exactly), then
  exp -> bf16 P^T tiles. PV for a two-block-lagged head interleaves per
  iteration in query-major form: out[128 queries, 65] = P^T-block
  (stationary) @ [V | 1] (moving) -- all 128 output partitions used, the
  denominator rides along as column 64, and each query-tile normalizes on
  DVE (reciprocal + per-partition scale) the moment its column finishes.
  The next pair's Q^T/K^T projection and, late in the run, the W_proj
  transposes are drip-fed between iterations as filler chunks.

  Tail: PV for heads 8 and 11, then attn-out is PE-transposed (bf16) to
  c-major and the output projection (bf16 x bf16) pipelines per n-tile
  with the transposes; bias add on DVE, stores on the sync queue.

Timing source: HW-calibrated cost-model timeline sim (no NTFF profiling
under this axon client). ~182.2 us/core vs 216.7 us for the v1 baseline;
rel err vs fp64 reference 5.5e-3 on hardware (bf16 operand rounding).
"""

import numpy as np

import concourse.bass as bass
import concourse.mybir as mybir
import concourse.tile as tile_mod
from concourse.alu_op_type import AluOpType
from concourse.masks import make_identity
from concourse.tile import TileContext


class TC(TileContext):
    """TileContext emitting at most one sync-wait per instruction.

    The pinned walrus rejects any instruction with >1 sem waits
    ("Too many sync wait commands", setupSyncWait), so excess waits are
    hoisted onto single-wait NoOps on the same engine right before the
    instruction, and the final drain is emitted as a drain chain.
    """

    _ww_counter = 0

    def _commit_instruction(self, inst, lazy_reg_writes: bool = True):
        si = getattr(inst, "sync_info", None)
        if si is not None and si.on_wait is not None and len(si.on_wait) > 1:
            waits = list(si.on_wait)
            for w in waits[:-1]:
                TC._ww_counter += 1
                nop = mybir.InstNoOp(
                    name=f"{inst.name}-ww{TC._ww_counter}",
                    engine=inst.engine,
                    sync_info=mybir.SyncInfo(on_wait=[w], on_update=[]),
                    bass_nofuse=True,
                )
                super()._commit_instruction(nop, lazy_reg_writes)
            inst.sync_info = mybir.SyncInfo(
                on_wait=waits[-1:], on_update=list(si.on_update))
        return super()._commit_instruction(inst, lazy_reg_writes)

    def _drain_and_barrier(self, tick_clock, wait_clock):
        drain_inst = self.nc.sync.drain()
        wait_clock.add_sem_waits(
            drain_inst.ins, tile_mod.ScopedClock({None: tick_clock.global_clock})
        )
        waits = list(drain_inst.ins.sync_info.on_wait)
        if len(waits) > 1:
            drain_inst.ins.sync_info = mybir.SyncInfo(on_wait=waits[:1], on_update=[])
            for w in waits[1:]:
                d2 = self.nc.sync.drain()
                d2.ins.sync_info = mybir.SyncInfo(on_wait=[w], on_update=[])
        self.nc.all_engine_barrier()
        assert self.sems is not None
        popped = self.nc._tile_sem_poison_stack.pop()
        assert popped is self._sem_poison
        self.nc.clear_and_free_semaphores(list(self.sems.allocated().values()))
        self.nc.all_engine_barrier()

N, C, H, HD = 1024, 768, 12, 64
B = 8
SCALE = HD ** -0.5
EPS = 1e-6
BIG = 1024.0          # mask bias magnitude (post-scale); exp(-1024) == 0
DVAL = 8192.0         # BIG / SCALE, exactly representable power of two
F32 = mybir.dt.float32
F32R = mybir.dt.float32r
BF16 = mybir.dt.bfloat16
AF = mybir.ActivationFunctionType
NT = N // 128      # 8 n-tiles
CT = C // 128      # 6 c-tiles
QKT = 2 * C // 128  # 12 c_out tiles for Q,K


def build_program():
    nc = bass.Bass()
    x_e = nc.declare_dram_parameter("x", [N, C], F32, isOutput=False)
    pol_e = nc.declare_dram_parameter("policy", [N, 1], F32, isOutput=False)
    wqkv_e = nc.declare_dram_parameter("w_qkv", [3 * C, C], F32, isOutput=False)
    wproj_e = nc.declare_dram_parameter("w_proj", [C, C], F32, isOutput=False)
    b_e = nc.declare_dram_parameter("b_proj", [C], F32, isOutput=False)
    out_e = nc.declare_dram_parameter("out", [N, C], F32, isOutput=True)

    with TC(nc) as tc:
        with tc.tile_pool(name="persist", bufs=1) as pp, \
             tc.tile_pool(name="psA", bufs=2, space="PSUM") as psA, \
             tc.tile_pool(name="psV1", bufs=1, space="PSUM") as psV1, \
             tc.tile_pool(name="psV2", bufs=1, space="PSUM") as psV2, \
             tc.tile_pool(name="psJ", bufs=1, space="PSUM") as psJ:

            # ---- constants ----
            ident = pp.tile([128, 128], F32, tag="ident")
            make_identity(nc, ident[:])
            ident_b = pp.tile([128, 128], BF16, tag="ident_b")
            nc.vector.tensor_copy(ident_b[:], ident[:])
            ident_r = pp.tile([128, 128], F32R, tag="ident_r")
            nc.vector.tensor_copy(ident_r[:], ident[:])
            pol_t = pp.tile([128, NT], F32, tag="pol")
            nc.sync.dma_start(out=pol_t[:], in_=pol_e.rearrange("(t p) o -> p (t o)", p=128))
            # Mask bias: (policy-1)*1024 -> 0 kept, -1024 dropped; exp(-1024)=0.
            logmask = pp.tile([128, NT], F32, tag="logmask")
            nc.vector.tensor_scalar(logmask[:], pol_t[:], -1.0, float(BIG),
                                    AluOpType.add, AluOpType.mult)
            # Diagonal unmask: add (1-policy[m])*2^13 to the raw score diagonal
            # so the ACT bias cancels exactly there (2^13 * SCALE == 1024).
            dpol = pp.tile([128, NT], F32, tag="dpol")
            nc.vector.tensor_scalar(dpol[:], pol_t[:], -1.0, -float(DVAL),
                                    AluOpType.add, AluOpType.mult)
            dmask = [pp.tile([128, 128], BF16, name=f"dmask{t}", tag=f"dmask{t}")
                     for t in range(NT)]
            for t in range(NT):
                nc.vector.tensor_scalar(dmask[t][:], ident[:], dpol[:, t:t + 1],
                                        None, AluOpType.mult)

            ones_f = pp.tile([128, H], F32, tag="ones_f")
            nc.gpsimd.memset(ones_f[:], 1.0)
            ones = pp.tile([128, H], BF16, tag="ones")
            nc.vector.tensor_copy(ones[:], ones_f[:])

            b_bc = pp.tile([128, C], F32, tag="b_bc")
            nc.sync.dma_start(
                out=b_bc[:],
                in_=b_e.rearrange("(o c) -> o c", o=1).to_broadcast([128, C]))

            # ---- persistent products ----
            vaug = [pp.tile([128, H * (HD + 1)], BF16, name=f"vaug{t}", tag=f"vaug{t}") for t in range(NT)]

            def cpy(i, out, in_):
                # alternate copies between DVE and ACT to halve the copy wall
                if i % 2 == 0:
                    nc.vector.tensor_copy(out, in_)
                else:
                    nc.scalar.copy(out, in_)

            def transpose(out, in_):
                nc.tensor.matmul(out, in_, ident_r[:], is_transpose=True,
                                 skip_group_check=True)

            # ---- phase 1: x transposes; V via JIT-transposed W_v ----
            with tc.tile_pool(name="loadL1", bufs=1) as l1p:
                xT = [l1p.tile([128, N], BF16, name=f"xT{c}", tag=f"xT{c}") for c in range(CT)]
                with tc.tile_pool(name="xrawp", bufs=1) as xrawp:
                    xraws = []
                    for nt in range(NT):
                        xr = xrawp.tile([128, C], F32R, name=f"xraw{nt}", tag=f"xraw{nt}")
                        nc.sync.dma_start(out=xr[:], in_=x_e[nt * 128:(nt + 1) * 128, :].bitcast(F32R))
                        xraws.append(xr)
                    for c in range(CT):
                        for half in range(2):
                            psg = psA.tile([128, N // 2], F32R, name="psg", tag="A")
                            for i in range(NT // 2):
                                nt = half * (NT // 2) + i
                                transpose(psg[:, i * 128:(i + 1) * 128],
                                          xraws[nt][:, c * 128:(c + 1) * 128])
                            cpy(c * 2 + half,
                                xT[c][:, half * (N // 2):(half + 1) * (N // 2)], psg[:])
                # pair-0 q/k rows load right behind x; wv after them
                qk0raw = {}
                for t0 in (0, CT):
                    wr0 = l1p.tile([128, C], F32R, name=f"qk0r{t0}", tag=f"qk0r{t0}")
                    nc.sync.dma_start(out=wr0[:], in_=wqkv_e[t0 * 128:(t0 + 1) * 128, :].bitcast(F32R))
                    qk0raw[t0] = wr0
                wvraws = []
                for i in range(CT):
                    rr = 2 * CT + i
                    wr = l1p.tile([128, C], F32R, name=f"wvraw{i}", tag=f"wvraw{i}")
                    nc.sync.dma_start(out=wr[:], in_=wqkv_e[rr * 128:(rr + 1) * 128, :].bitcast(F32R))
                    wvraws.append(wr)
                wvT = [l1p.tile([128, C], BF16, name=f"wvT{c}", tag=f"wvT{c}") for c in range(CT)]
                for nt in range(NT):
                    nc.vector.tensor_copy(
                        vaug[nt][:].rearrange("p (h e) -> p e h", e=HD + 1)[:, HD:HD + 1, :],
                        ones[:, 0:H].rearrange("p (o h) -> p o h", o=1))

                # ---- phase 3: attention with JIT qkv^T between heads ----
                with tc.tile_pool(name="loadL2", bufs=1) as l2p, \
                         tc.tile_pool(name="rawcyc2", bufs=1) as rawp2, \
                         tc.tile_pool(name="jitq", bufs=2) as jitq, \
                         tc.tile_pool(name="qkp", bufs=4) as qkp, \
                         tc.tile_pool(name="work", bufs=2) as wp, \
                         tc.tile_pool(name="ptp", bufs=24) as ptp, \
                         tc.tile_pool(name="epi", bufs=3) as ep:
                    awide = [l2p.tile([128, C], BF16, name=f"aw{t}", tag=f"aw{t}")
                             for t in range(NT)]
                    attnT = l2p.tile([128, CT * N], BF16, tag="attnT")

                    wpraws = []
                    wpT = [l2p.tile([128, C], BF16, name=f"wpT{p}", tag=f"wpT{p}")
                           for p in range(H // 2)]

                    def wpt_chunks():
                        for hp in range(H // 2):
                            def tchunk(hp=hp):
                                psg = psJ.tile([128, C], F32R, name="psg3", tag="J")
                                for rr in range(CT):
                                    transpose(psg[:, rr * 128:(rr + 1) * 128],
                                              wpraws[rr][:, hp * 128:(hp + 1) * 128])
                                cpy(hp, wpT[hp][:], psg[:])
                            yield tchunk

                    def load_wproj():
                        # deferred prefetch: issued mid-attention so it never
                        # competes with the x/w_qkv loads for DMA bandwidth
                        for rr in range(CT):
                            wpraw = rawp2.tile([128, C], F32R, name=f"wpraw{rr}", tag=f"wpraw{rr}")
                            nc.sync.dma_start(out=wpraw[:], in_=wproj_e[rr * 128:(rr + 1) * 128, :].bitcast(F32R))
                            wpraws.append(wpraw)

                    qkT = {}

                    def jit_chunks(t, pool=None):
                        """Chunked Q^T/K^T projection for w_qkv row-tile t."""
                        pool = pool or psJ
                        tag = "J" if pool is psJ else "A"
                        if t in qk0raw:
                            wqr = qk0raw[t]
                        else:
                            wqr = jitq.tile([128, C], F32R, name="wqr", tag="wqr")
                            nc.sync.dma_start(out=wqr[:], in_=wqkv_e[t * 128:(t + 1) * 128, :].bitcast(F32R))
                        psg = pool.tile([128, C], F32R, name="psgq", tag=tag)
                        def tchunk():
                            for c in range(CT):
                                transpose(psg[:, c * 128:(c + 1) * 128],
                                          wqr[:, c * 128:(c + 1) * 128])
                        yield tchunk
                        wqTt = jitq.tile([128, C], BF16, name="wqTt", tag="wqTt")
                        yield lambda: nc.vector.tensor_copy(wqTt[:], psg[:])
                        psq = pool.tile([128, N], F32, name="psJN", tag=tag)
                        for c0 in range(0, CT, 2):
                            for j in range(2):
                                def mm(c0=c0, j=j):
                                    for c in (c0, c0 + 1):
                                        nc.tensor.matmul(
                                            psq[:, j * 512:(j + 1) * 512],
                                            wqTt[:, c * 128:(c + 1) * 128],
                                            xT[c][:, j * 512:(j + 1) * 512],
                                            start=(c == 0), stop=(c == CT - 1),
                                            skip_group_check=True)
                                yield mm
                        def fin():
                            qo = qkp.tile([128, N], BF16, name="qkvTt", tag="qk")
                            nc.vector.tensor_copy(qo[:], psq[:])
                            qkT[t] = qo
                        yield fin

                    def wvt_chunks():
                        """W_v^T via PE transposes (chunked)."""
                        for c in range(CT):
                            def tchunk(c=c):
                                psg = psA.tile([128, C], F32R, name="psg2", tag="A")
                                for i in range(CT):
                                    transpose(psg[:, i * 128:(i + 1) * 128],
                                              wvraws[i][:, c * 128:(c + 1) * 128])
                                cpy(c, wvT[c][:], psg[:])
                            yield tchunk

                    def v_chunks(nt):
                        """V projection for n-tile nt into the (idle until the
                        first PV) psV banks; finishes with the bf16 vaug copy."""
                        a = psV1.tile([128, 512], F32, name=f"vA{nt}", tag="V1")
                        bt = psV2.tile([128, 512], F32, name=f"vB{nt}", tag="V2")
                        for c in range(CT):
                            def mm(c=c):
                                nc.tensor.matmul(
                                    a[:, 0:512],
                                    xT[c][:, nt * 128:(nt + 1) * 128],
                                    wvT[c][:, 0:512],
                                    start=(c == 0), stop=(c == CT - 1),
                                    skip_group_check=True)
                                nc.tensor.matmul(
                                    bt[:, 0:256],
                                    xT[c][:, nt * 128:(nt + 1) * 128],
                                    wvT[c][:, 512:768],
                                    start=(c == 0), stop=(c == CT - 1),
                                    skip_group_check=True)
                            yield mm
                        def fin():
                            nc.vector.tensor_copy(
                                vaug[nt][:].rearrange("p (h e) -> p h e", h=H)[:, 0:8, 0:HD],
                                a[:, 0:512].rearrange("p (h e) -> p h e", h=8))
                            nc.vector.tensor_copy(
                                vaug[nt][:].rearrange("p (h e) -> p h e", h=H)[:, 8:12, 0:HD],
                                bt[:, 0:256].rearrange("p (h e) -> p h e", h=4))
                        yield fin

                    filler = []

                    def pop_filler(budget):
                        done = 0
                        while filler and done < budget:
                            try:
                                next(filler[0])()
                                done += 1
                            except StopIteration:
                                filler.pop(0)

                    ptiles = {}   # h -> [8 bf16 P^T tiles]
                    psv = {}      # h -> (psum nt 0-3, psum nt 4-7)

                    def psv_alloc(h, pool="V"):
                        if pool == "V":
                            psv[h] = (
                                psV1.tile([128, 512], F32, name=f"pva{h}", tag="V1"),
                                psV2.tile([128, 512], F32, name=f"pvb{h}", tag="V2"))
                        else:
                            # the JIT psum bank is free late in the run; host a
                            # second concurrent PV stream there
                            tile = psJ.tile([128, N], F32, name=f"pvj{h}", tag="J")
                            psv[h] = (tile[:, 0:512], tile[:, 512:1024])

                    def emit_head(h, qt, kt, pv_hs):
                        """Scores+exp for head h; PV matmuls for the lagged
                        heads in pv_hs interleaved per key-tile, each
                        normalized per query-tile as its column finishes."""
                        rb = (h % 2) * 64
                        for i, pv_h in enumerate(pv_hs):
                            psv_alloc(pv_h, "V" if i == 0 else "J")
                        for mt in range(NT):
                            ps = psA.tile([128, N], F32, name="psN", tag="A")
                            for j in range(2):
                                nc.tensor.matmul(
                                    ps[:, j * 512:(j + 1) * 512],
                                    kt[rb:rb + HD, mt * 128:(mt + 1) * 128],
                                    qt[rb:rb + HD, j * 512:(j + 1) * 512],
                                    start=True, stop=(j != mt // 4),
                                    skip_group_check=True)
                            nc.tensor.matmul(
                                ps[:, mt * 128:(mt + 1) * 128],
                                ident_b[:], dmask[mt][:],
                                start=False, stop=True, skip_group_check=True)
                            ptile = ptp.tile([128, N], BF16, name=f"pt{h}_{mt}", tag="pt")
                            nc.scalar.activation(ptile[:], ps[:], AF.Exp,
                                                 bias=logmask[:, mt:mt + 1], scale=SCALE)
                            ptiles.setdefault(h, []).append(ptile)
                            for pv_h in pv_hs:
                                emit_pv_nt(pv_h, mt)
                                emit_norm_nt(pv_h, mt)
                            pop_filler(2 if pv_hs else 8)
                        for pv_h in pv_hs:
                            del psv[pv_h]
                            del ptiles[pv_h]

                    def emit_pv_nt(h, nt):
                        # query-major PV for one query tile: stationary P^T
                        # block [128 keys, 128 queries], moving [V | 1]
                        # [128 keys, 65] -> psum [128 queries, 65] with the
                        # denominator in col 64. The 8 key-tile matmuls are
                        # contiguous: one PSUM bank allows only one open
                        # accumulation group at a time.
                        a, bt = psv[h]
                        tgt = a if nt < 4 else bt
                        off = (nt % 4) * 128
                        for mt in range(NT):
                            nc.tensor.matmul(
                                tgt[:, off:off + HD + 1],
                                ptiles[h][mt][:, nt * 128:(nt + 1) * 128],
                                vaug[mt][:, h * (HD + 1):(h + 1) * (HD + 1)],
                                start=(mt == 0), stop=(mt == NT - 1),
                                skip_group_check=True)

                    dens = {}

                    def emit_norm_nt(h, nt):
                        # normalize query-tile nt of head h right after its PV
                        # column finishes, so the next block's PV stream never
                        # waits on a batched norm chain
                        a, bt = psv[h]
                        t = a if nt < 4 else bt
                        off = (nt % 4) * 128
                        if nt == 0:
                            dens[h] = ep.tile([128, NT], F32, name=f"den{h}", tag="den")
                        den = dens[h]
                        nc.vector.tensor_scalar_add(
                            den[:, nt:nt + 1], t[:, off + HD:off + HD + 1], EPS)
                        nc.vector.reciprocal(den[:, nt:nt + 1], den[:, nt:nt + 1])
                        nc.vector.tensor_scalar(
                            awide[nt][:, h * HD:(h + 1) * HD],
                            t[:, off:off + HD], den[:, nt:nt + 1], None,
                            AluOpType.mult)

                    # pair-0 q/k projected up-front (before V, right after
                    # xT), round-robin so the q and k chains overlap across
                    # their two psum pools
                    g0, g1 = jit_chunks(0), jit_chunks(CT, pool=psA)
                    d0 = d1 = False
                    while not (d0 and d1):
                        if not d0:
                            try:
                                next(g0)()
                            except StopIteration:
                                d0 = True
                        if not d1:
                            try:
                                next(g1)()
                            except StopIteration:
                                d1 = True
                    # wvT + all of V run as filler inside the first two score
                    # blocks (heads 1 and 0); PV lags its head by two blocks.
                    filler.append(wvt_chunks())
                    for nt in range(NT):
                        filler.append(v_chunks(nt))

                    seq = []
                    for tp in range(CT):
                        seq += [2 * tp + 1, 2 * tp]
                    seq[10], seq[11] = 10, 11   # last pair even-first: PV(10)
                    # then rides block 11 and only PV(11) trails the loop
                    for bi, h in enumerate(seq):
                        tp = h // 2
                        if h % 2 == 1 and tp + 1 < CT:
                            filler.append(jit_chunks(tp + 1))
                            filler.append(jit_chunks(CT + tp + 1))
                        if h % 2 == 1 and (tp not in qkT or CT + tp not in qkT):
                            # this pair's q/k must be fully emitted first
                            while tp not in qkT or CT + tp not in qkT:
                                pop_filler(1)
                        if bi == 6:
                            load_wproj()
                        if bi == 8:
                            filler.append(wpt_chunks())
                        pvs = [(), (), (1,), (0,), (3,), (2,), (5,), (4,),
                               (7,), (6,), (9,), (10,)]
                        emit_head(h, qkT[tp], qkT[CT + tp], pvs[bi])

                    # trailing PVs: h8 (ready immediately, on the psJ bank)
                    # and h11 (gated on the final block's exps)
                    psv_alloc(8, "J")
                    for nt in range(NT):
                        emit_pv_nt(8, nt)
                        emit_norm_nt(8, nt)
                    del psv[8]
                    del ptiles[8]
                    psv_alloc(11, "V")

                    # attn-out -> c-major PE transposes pipelined with the
                    # projection: transpose(nt+1) hides proj(nt)'s wait on the
                    # attnT copy. (block (c, nt) of attnT at col c*N + nt*128)
                    def emit_trans(nt):
                        psg = psJ.tile([128, C], BF16, name="psgA", tag="J")
                        for c in range(CT):
                            nc.tensor.matmul(psg[:, c * 128:(c + 1) * 128],
                                             awide[nt][:, c * 128:(c + 1) * 128],
                                             ident_b[:], is_transpose=True,
                                             skip_group_check=True)
                        cpy(nt, attnT[:].rearrange("p (t nn) -> p t nn", t=CT)[
                            :, :, nt * 128:nt * 128 + 128], psg[:].rearrange(
                            "p (t nn) -> p t nn", t=CT))

                    def emit_proj(nt):
                        ps = psA.tile([128, C], F32, name="psC2", tag="A")
                        for hp in range(H // 2):
                            for f0, fw in ((0, 512), (512, 256)):
                                nc.tensor.matmul(
                                    ps[:, f0:f0 + fw],
                                    attnT[:, hp * N + nt * 128: hp * N + nt * 128 + 128],
                                    wpT[hp][:, f0:f0 + fw],
                                    start=(hp == 0), stop=(hp == H // 2 - 1))
                        yt = wp.tile([128, C], F32, name="yt", tag="yt")
                        nc.vector.tensor_tensor(yt[:], ps[:], b_bc[:], AluOpType.add)
                        nc.sync.dma_start(out=out_e[nt * 128:(nt + 1) * 128, :], in_=yt[:])

                    for nt in range(NT):
                        emit_pv_nt(11, nt)
                        emit_norm_nt(11, nt)
                    del psv[11]
                    del ptiles[11]
                    emit_trans(0)
                    for nt in range(1, NT):
                        emit_trans(nt)
                        emit_proj(nt - 1)
                    emit_proj(NT - 1)

    return nc


_NC = None


def _get_nc():
    global _NC
    if _NC is None:
        _NC = build_program()
    return _NC


def run(in_maps, trace=False, **kw):
    from concourse.bass_utils import run_bass_kernel_spmd
    return run_bass_kernel_spmd(_get_nc(), in_maps, core_ids=list(range(B)),
                                trace=trace, **kw)


def kernel(x, policy, w_qkv, w_proj, b_proj):
    x = np.ascontiguousarray(np.asarray(x, dtype=np.float32))
    policy = np.ascontiguousarray(np.asarray(policy, dtype=np.float32))
    w_qkv = np.ascontiguousarray(np.asarray(w_qkv, dtype=np.float32))
    w_proj = np.ascontiguousarray(np.asarray(w_proj, dtype=np.float32))
    b_proj = np.ascontiguousarray(np.asarray(b_proj, dtype=np.float32))
    in_maps = [
        {"x": x[i], "policy": policy[i], "w_qkv": w_qkv,
         "w_proj": w_proj, "b_proj": b_proj}
        for i in range(B)
    ]
    try:
        res = run(in_maps)
    except Exception:
        # one observed transient NRT_EXEC_UNIT_UNRECOVERABLE wedge in ~40
        # invocations this session; a retry is free insurance
        res = run(in_maps)
    return np.stack([res.results[i]["out"] for i in range(B)], axis=0)


if __name__ == "__main__":
    rng = np.random.default_rng(0)
    x = rng.standard_normal((B, N, C), dtype=np.float32)
    policy = (rng.random((B, N, 1)) > 0.3).astype(np.float32)
    w_qkv = rng.standard_normal((3 * C, C), dtype=np.float32) * C ** -0.5
    w_proj = rng.standard_normal((C, C), dtype=np.float32) * C ** -0.5
    b_proj = np.zeros((C,), dtype=np.float32)
    y = kernel(x=x, policy=policy, w_qkv=w_qkv, w_proj=w_proj, b_proj=b_proj)
    print("out", y.shape, y.dtype, np.abs(y).mean())



# revision 42
# speedup vs baseline: 1.0716x; 1.0059x over previous
"""Sparse (policy-masked) attention on 8 TRN2 NeuronCores.

Data-parallel over batch (B=8 -> one batch element per core, weights
replicated, no collectives). Per core:

  Prologue: x and the pair-0 q/k weight rows load first; x is PE-transposed
  (f32r), pair-0 Q^T/K^T are projected immediately (q on the JIT psum bank,
  k on the score bank) so the exp stream starts ~20us in. W_v transposes and
  the V projection run as filler chunks inside the first two score blocks,
  with V accumulating in the then-idle PV psum banks.

  Attention: heads are processed in pair order [1,0,3,2,...,10,11]. Per
  key-tile iteration: S^T = K Q^T (bf16, key-major so the policy mask is a
  per-partition ACT bias), a bf16 identity matmul restores the always-keep
  diagonal (+2^13 pre-scale cancels the -1024 mask bias exactly), then
  exp -> bf16 P^T tiles. PV for a two-block-lagged head interleaves per
  iteration in query-major form: out[128 queries, 65] = P^T-block
  (stationary) @ [V | 1] (moving) -- all 128 output partitions used, the
  denominator rides along as column 64, and each query-tile normalizes on
  DVE (reciprocal + per-partition scale) the moment its column finishes.
  The next pair's Q^T/K^T projection and, late in the run, the W_proj
  transposes are drip-fed between iterations as filler chunks.

  Tail: PV for heads 8 and 11, then attn-out is PE-transposed (bf16) to
  c-major and the output projection (bf16 x bf16) pipelines per n-tile
  with the transposes; bias add on DVE, stores on the sync queue.

Timing source: HW-calibrated cost-model timeline sim (no NTFF profiling
under this axon client). ~181.1 us/core vs 216.7 us for the v1 baseline;
rel err vs fp64 reference 5.5e-3 on hardware (bf16 operand rounding).
"""

import numpy as np

import concourse.bass as bass
import concourse.mybir as mybir
import concourse.tile as tile_mod
from concourse.alu_op_type import AluOpType
from concourse.masks import make_identity
from concourse.tile import TileContext


class TC(TileContext):
    """TileContext emitting at most one sync-wait per instruction.

    The pinned walrus rejects any instruction with >1 sem waits
    ("Too many sync wait commands", setupSyncWait), so excess waits are
    hoisted onto single-wait NoOps on the same engine right before the
    instruction, and the final drain is emitted as a drain chain.
    """

    _ww_counter = 0

    def _commit_instruction(self, inst, lazy_reg_writes: bool = True):
        si = getattr(inst, "sync_info", None)
        if si is not None and si.on_wait is not None and len(si.on_wait) > 1:
            waits = list(si.on_wait)
            for w in waits[:-1]:
                TC._ww_counter += 1
                nop = mybir.InstNoOp(
                    name=f"{inst.name}-ww{TC._ww_counter}",
                    engine=inst.engine,
                    sync_info=mybir.SyncInfo(on_wait=[w], on_update=[]),
                    bass_nofuse=True,
                )
                super()._commit_instruction(nop, lazy_reg_writes)
            inst.sync_info = mybir.SyncInfo(
                on_wait=waits[-1:], on_update=list(si.on_update))
        return super()._commit_instruction(inst, lazy_reg_writes)

    def _drain_and_barrier(self, tick_clock, wait_clock):
        drain_inst = self.nc.sync.drain()
        wait_clock.add_sem_waits(
            drain_inst.ins, tile_mod.ScopedClock({None: tick_clock.global_clock})
        )
        waits = list(drain_inst.ins.sync_info.on_wait)
        if len(waits) > 1:
            drain_inst.ins.sync_info = mybir.SyncInfo(on_wait=waits[:1], on_update=[])
            for w in waits[1:]:
                d2 = self.nc.sync.drain()
                d2.ins.sync_info = mybir.SyncInfo(on_wait=[w], on_update=[])
        self.nc.all_engine_barrier()
        assert self.sems is not None
        popped = self.nc._tile_sem_poison_stack.pop()
        assert popped is self._sem_poison
        self.nc.clear_and_free_semaphores(list(self.sems.allocated().values()))
        self.nc.all_engine_barrier()

N, C, H, HD = 1024, 768, 12, 64
B = 8
SCALE = HD ** -0.5
EPS = 1e-6
BIG = 1024.0          # mask bias magnitude (post-scale); exp(-1024) == 0
DVAL = 8192.0         # BIG / SCALE, exactly representable power of two
F32 = mybir.dt.float32
F32R = mybir.dt.float32r
BF16 = mybir.dt.bfloat16
AF = mybir.ActivationFunctionType
NT = N // 128      # 8 n-tiles
CT = C // 128      # 6 c-tiles
QKT = 2 * C // 128  # 12 c_out tiles for Q,K


def build_program():
    nc = bass.Bass()
    x_e = nc.declare_dram_parameter("x", [N, C], F32, isOutput=False)
    pol_e = nc.declare_dram_parameter("policy", [N, 1], F32, isOutput=False)
    wqkv_e = nc.declare_dram_parameter("w_qkv", [3 * C, C], F32, isOutput=False)
    wproj_e = nc.declare_dram_parameter("w_proj", [C, C], F32, isOutput=False)
    b_e = nc.declare_dram_parameter("b_proj", [C], F32, isOutput=False)
    out_e = nc.declare_dram_parameter("out", [N, C], F32, isOutput=True)

    with TC(nc) as tc:
        with tc.tile_pool(name="persist", bufs=1) as pp, \
             tc.tile_pool(name="psA", bufs=2, space="PSUM") as psA, \
             tc.tile_pool(name="psV1", bufs=1, space="PSUM") as psV1, \
             tc.tile_pool(name="psV2", bufs=1, space="PSUM") as psV2, \
             tc.tile_pool(name="psJ", bufs=1, space="PSUM") as psJ:

            # ---- constants ----
            ident = pp.tile([128, 128], F32, tag="ident")
            make_identity(nc, ident[:])
            ident_b = pp.tile([128, 128], BF16, tag="ident_b")
            nc.vector.tensor_copy(ident_b[:], ident[:])
            ident_r = pp.tile([128, 128], F32R, tag="ident_r")
            nc.vector.tensor_copy(ident_r[:], ident[:])
            pol_t = pp.tile([128, NT], F32, tag="pol")
            nc.sync.dma_start(out=pol_t[:], in_=pol_e.rearrange("(t p) o -> p (t o)", p=128))
            # Mask bias: (policy-1)*1024 -> 0 kept, -1024 dropped; exp(-1024)=0.
            logmask = pp.tile([128, NT], F32, tag="logmask")
            nc.vector.tensor_scalar(logmask[:], pol_t[:], -1.0, float(BIG),
                                    AluOpType.add, AluOpType.mult)
            # Diagonal unmask: add (1-policy[m])*2^13 to the raw score diagonal
            # so the ACT bias cancels exactly there (2^13 * SCALE == 1024).
            dpol = pp.tile([128, NT], F32, tag="dpol")
            nc.vector.tensor_scalar(dpol[:], pol_t[:], -1.0, -float(DVAL),
                                    AluOpType.add, AluOpType.mult)
            dmask = [pp.tile([128, 128], BF16, name=f"dmask{t}", tag=f"dmask{t}")
                     for t in range(NT)]
            for t in range(NT):
                nc.vector.tensor_scalar(dmask[t][:], ident[:], dpol[:, t:t + 1],
                                        None, AluOpType.mult)

            ones_f = pp.tile([128, H], F32, tag="ones_f")
            nc.gpsimd.memset(ones_f[:], 1.0)
            ones = pp.tile([128, H], BF16, tag="ones")
            nc.vector.tensor_copy(ones[:], ones_f[:])

            b_bc = pp.tile([128, C], F32, tag="b_bc")
            nc.sync.dma_start(
                out=b_bc[:],
                in_=b_e.rearrange("(o c) -> o c", o=1).to_broadcast([128, C]))

            # ---- persistent products ----
            vaug = [pp.tile([128, H * (HD + 1)], BF16, name=f"vaug{t}", tag=f"vaug{t}") for t in range(NT)]

            def cpy(i, out, in_):
                # alternate copies between DVE and ACT to halve the copy wall
                if i % 2 == 0:
                    nc.vector.tensor_copy(out, in_)
                else:
                    nc.scalar.copy(out, in_)

            def transpose(out, in_):
                nc.tensor.matmul(out, in_, ident_r[:], is_transpose=True,
                                 skip_group_check=True)

            # ---- phase 1: x transposes; V via JIT-transposed W_v ----
            with tc.tile_pool(name="loadL1", bufs=1) as l1p:
                xT = [l1p.tile([128, N], BF16, name=f"xT{c}", tag=f"xT{c}") for c in range(CT)]
                with tc.tile_pool(name="xrawp", bufs=1) as xrawp:
                    xraws = []
                    for nt in range(NT):
                        xr = xrawp.tile([128, C], F32R, name=f"xraw{nt}", tag=f"xraw{nt}")
                        nc.sync.dma_start(out=xr[:], in_=x_e[nt * 128:(nt + 1) * 128, :].bitcast(F32R))
                        xraws.append(xr)
                    for c in range(CT):
                        for half in range(2):
                            psg = psA.tile([128, N // 2], F32R, name="psg", tag="A")
                            for i in range(NT // 2):
                                nt = half * (NT // 2) + i
                                transpose(psg[:, i * 128:(i + 1) * 128],
                                          xraws[nt][:, c * 128:(c + 1) * 128])
                            nc.scalar.copy(
                                xT[c][:, half * (N // 2):(half + 1) * (N // 2)], psg[:])
                # pair-0 q/k rows load right behind x; wv after them
                qk0raw = {}
                for t0 in (0, CT):
                    wr0 = l1p.tile([128, C], F32R, name=f"qk0r{t0}", tag=f"qk0r{t0}")
                    nc.sync.dma_start(out=wr0[:], in_=wqkv_e[t0 * 128:(t0 + 1) * 128, :].bitcast(F32R))
                    qk0raw[t0] = wr0
                wvraws = []
                for i in range(CT):
                    rr = 2 * CT + i
                    wr = l1p.tile([128, C], F32R, name=f"wvraw{i}", tag=f"wvraw{i}")
                    nc.sync.dma_start(out=wr[:], in_=wqkv_e[rr * 128:(rr + 1) * 128, :].bitcast(F32R))
                    wvraws.append(wr)
                wvT = [l1p.tile([128, C], BF16, name=f"wvT{c}", tag=f"wvT{c}") for c in range(CT)]
                for nt in range(NT):
                    nc.vector.tensor_copy(
                        vaug[nt][:].rearrange("p (h e) -> p e h", e=HD + 1)[:, HD:HD + 1, :],
                        ones[:, 0:H].rearrange("p (o h) -> p o h", o=1))

                # ---- phase 3: attention with JIT qkv^T between heads ----
                with tc.tile_pool(name="loadL2", bufs=1) as l2p, \
                         tc.tile_pool(name="rawcyc2", bufs=1) as rawp2, \
                         tc.tile_pool(name="jitq", bufs=2) as jitq, \
                         tc.tile_pool(name="qkp", bufs=4) as qkp, \
                         tc.tile_pool(name="work", bufs=2) as wp, \
                         tc.tile_pool(name="ptp", bufs=24) as ptp, \
                         tc.tile_pool(name="epi", bufs=3) as ep:
                    awide = [l2p.tile([128, C], BF16, name=f"aw{t}", tag=f"aw{t}")
                             for t in range(NT)]
                    attnT = l2p.tile([128, CT * N], BF16, tag="attnT")

                    wpraws = []
                    wpT = [l2p.tile([128, C], BF16, name=f"wpT{p}", tag=f"wpT{p}")
                           for p in range(H // 2)]

                    def wpt_chunks():
                        for hp in range(H // 2):
                            def tchunk(hp=hp):
                                psg = psJ.tile([128, C], F32R, name="psg3", tag="J")
                                for rr in range(CT):
                                    transpose(psg[:, rr * 128:(rr + 1) * 128],
                                              wpraws[rr][:, hp * 128:(hp + 1) * 128])
                                cpy(hp, wpT[hp][:], psg[:])
                            yield tchunk

                    def load_wproj():
                        # deferred prefetch: issued mid-attention so it never
                        # competes with the x/w_qkv loads for DMA bandwidth
                        for rr in range(CT):
                            wpraw = rawp2.tile([128, C], F32R, name=f"wpraw{rr}", tag=f"wpraw{rr}")
                            nc.sync.dma_start(out=wpraw[:], in_=wproj_e[rr * 128:(rr + 1) * 128, :].bitcast(F32R))
                            wpraws.append(wpraw)

                    qkT = {}

                    def jit_chunks(t, pool=None):
                        """Chunked Q^T/K^T projection for w_qkv row-tile t."""
                        pool = pool or psJ
                        tag = "J" if pool is psJ else "A"
                        if t in qk0raw:
                            wqr = qk0raw[t]
                        else:
                            wqr = jitq.tile([128, C], F32R, name="wqr", tag="wqr")
                            nc.sync.dma_start(out=wqr[:], in_=wqkv_e[t * 128:(t + 1) * 128, :].bitcast(F32R))
                        psg = pool.tile([128, C], F32R, name="psgq", tag=tag)
                        def tchunk():
                            for c in range(CT):
                                transpose(psg[:, c * 128:(c + 1) * 128],
                                          wqr[:, c * 128:(c + 1) * 128])
                        yield tchunk
                        wqTt = jitq.tile([128, C], BF16, name="wqTt", tag="wqTt")
                        yield lambda: nc.vector.tensor_copy(wqTt[:], psg[:])
                        psq = pool.tile([128, N], F32, name="psJN", tag=tag)
                        for c0 in range(0, CT, 2):
                            for j in range(2):
                                def mm(c0=c0, j=j):
                                    for c in (c0, c0 + 1):
                                        nc.tensor.matmul(
                                            psq[:, j * 512:(j + 1) * 512],
                                            wqTt[:, c * 128:(c + 1) * 128],
                                            xT[c][:, j * 512:(j + 1) * 512],
                                            start=(c == 0), stop=(c == CT - 1),
                                            skip_group_check=True)
                                yield mm
                        def fin():
                            qo = qkp.tile([128, N], BF16, name="qkvTt", tag="qk")
                            nc.vector.tensor_copy(qo[:], psq[:])
                            qkT[t] = qo
                        yield fin

                    def wvt_chunks():
                        """W_v^T via PE transposes (chunked)."""
                        for c in range(CT):
                            def tchunk(c=c):
                                psg = psA.tile([128, C], F32R, name="psg2", tag="A")
                                for i in range(CT):
                                    transpose(psg[:, i * 128:(i + 1) * 128],
                                              wvraws[i][:, c * 128:(c + 1) * 128])
                                cpy(c, wvT[c][:], psg[:])
                            yield tchunk

                    def v_chunks(nt):
                        """V projection for n-tile nt into the (idle until the
                        first PV) psV banks; finishes with the bf16 vaug copy."""
                        a = psV1.tile([128, 512], F32, name=f"vA{nt}", tag="V1")
                        bt = psV2.tile([128, 512], F32, name=f"vB{nt}", tag="V2")
                        for c in range(CT):
                            def mm(c=c):
                                nc.tensor.matmul(
                                    a[:, 0:512],
                                    xT[c][:, nt * 128:(nt + 1) * 128],
                                    wvT[c][:, 0:512],
                                    start=(c == 0), stop=(c == CT - 1),
                                    skip_group_check=True)
                                nc.tensor.matmul(
                                    bt[:, 0:256],
                                    xT[c][:, nt * 128:(nt + 1) * 128],
                                    wvT[c][:, 512:768],
                                    start=(c == 0), stop=(c == CT - 1),
                                    skip_group_check=True)
                            yield mm
                        def fin():
                            nc.vector.tensor_copy(
                                vaug[nt][:].rearrange("p (h e) -> p h e", h=H)[:, 0:8, 0:HD],
                                a[:, 0:512].rearrange("p (h e) -> p h e", h=8))
                            nc.vector.tensor_copy(
                                vaug[nt][:].rearrange("p (h e) -> p h e", h=H)[:, 8:12, 0:HD],
                                bt[:, 0:256].rearrange("p (h e) -> p h e", h=4))
                        yield fin

                    filler = []

                    def pop_filler(budget):
                        done = 0
                        while filler and done < budget:
                            try:
                                next(filler[0])()
                                done += 1
                            except StopIteration:
                                filler.pop(0)

                    ptiles = {}   # h -> [8 bf16 P^T tiles]
                    psv = {}      # h -> (psum nt 0-3, psum nt 4-7)

                    def psv_alloc(h, pool="V"):
                        if pool == "V":
                            psv[h] = (
                                psV1.tile([128, 512], F32, name=f"pva{h}", tag="V1"),
                                psV2.tile([128, 512], F32, name=f"pvb{h}", tag="V2"))
                        else:
                            # the JIT psum bank is free late in the run; host a
                            # second concurrent PV stream there
                            tile = psJ.tile([128, N], F32, name=f"pvj{h}", tag="J")
                            psv[h] = (tile[:, 0:512], tile[:, 512:1024])

                    def emit_head(h, qt, kt, pv_hs):
                        """Scores+exp for head h; PV matmuls for the lagged
                        heads in pv_hs interleaved per key-tile, each
                        normalized per query-tile as its column finishes."""
                        rb = (h % 2) * 64
                        for i, pv_h in enumerate(pv_hs):
                            psv_alloc(pv_h, "V" if i == 0 else "J")
                        for mt in range(NT):
                            ps = psA.tile([128, N], F32, name="psN", tag="A")
                            for j in range(2):
                                nc.tensor.matmul(
                                    ps[:, j * 512:(j + 1) * 512],
                                    kt[rb:rb + HD, mt * 128:(mt + 1) * 128],
                                    qt[rb:rb + HD, j * 512:(j + 1) * 512],
                                    start=True, stop=(j != mt // 4),
                                    skip_group_check=True)
                            nc.tensor.matmul(
                                ps[:, mt * 128:(mt + 1) * 128],
                                ident_b[:], dmask[mt][:],
                                start=False, stop=True, skip_group_check=True)
                            ptile = ptp.tile([128, N], BF16, name=f"pt{h}_{mt}", tag="pt")
                            nc.scalar.activation(ptile[:], ps[:], AF.Exp,
                                                 bias=logmask[:, mt:mt + 1], scale=SCALE)
                            ptiles.setdefault(h, []).append(ptile)
                            for pv_h in pv_hs:
                                emit_pv_nt(pv_h, mt)
                                emit_norm_nt(pv_h, mt)
                            pop_filler(2 if pv_hs else 8)
                        for pv_h in pv_hs:
                            del psv[pv_h]
                            del ptiles[pv_h]

                    def emit_pv_nt(h, nt):
                        # query-major PV for one query tile: stationary P^T
                        # block [128 keys, 128 queries], moving [V | 1]
                        # [128 keys, 65] -> psum [128 queries, 65] with the
                        # denominator in col 64. The 8 key-tile matmuls are
                        # contiguous: one PSUM bank allows only one open
                        # accumulation group at a time.
                        a, bt = psv[h]
                        tgt = a if nt < 4 else bt
                        off = (nt % 4) * 128
                        for mt in range(NT):
                            nc.tensor.matmul(
                                tgt[:, off:off + HD + 1],
                                ptiles[h][mt][:, nt * 128:(nt + 1) * 128],
                                vaug[mt][:, h * (HD + 1):(h + 1) * (HD + 1)],
                                start=(mt == 0), stop=(mt == NT - 1),
                                skip_group_check=True)

                    dens = {}

                    def emit_norm_nt(h, nt):
                        # normalize query-tile nt of head h right after its PV
                        # column finishes, so the next block's PV stream never
                        # waits on a batched norm chain
                        a, bt = psv[h]
                        t = a if nt < 4 else bt
                        off = (nt % 4) * 128
                        if nt == 0:
                            dens[h] = ep.tile([128, NT], F32, name=f"den{h}", tag="den")
                        den = dens[h]
                        nc.vector.tensor_scalar_add(
                            den[:, nt:nt + 1], t[:, off + HD:off + HD + 1], EPS)
                        nc.vector.reciprocal(den[:, nt:nt + 1], den[:, nt:nt + 1])
                        nc.vector.tensor_scalar(
                            awide[nt][:, h * HD:(h + 1) * HD],
                            t[:, off:off + HD], den[:, nt:nt + 1], None,
                            AluOpType.mult)

                    # pair-0 q/k projected up-front (before V, right after
                    # xT), round-robin so the q and k chains overlap across
                    # their two psum pools
                    g0, g1 = jit_chunks(0), jit_chunks(CT, pool=psA)
                    d0 = d1 = False
                    while not (d0 and d1):
                        if not d0:
                            try:
                                next(g0)()
                            except StopIteration:
                                d0 = True
                        if not d1:
                            try:
                                next(g1)()
                            except StopIteration:
                                d1 = True
                    # wvT + all of V run as filler inside the first two score
                    # blocks (heads 1 and 0); PV lags its head by two blocks.
                    filler.append(wvt_chunks())
                    for nt in range(NT):
                        filler.append(v_chunks(nt))

                    seq = []
                    for tp in range(CT):
                        seq += [2 * tp + 1, 2 * tp]
                    seq[10], seq[11] = 10, 11   # last pair even-first: PV(10)
                    # then rides block 11 and only PV(11) trails the loop
                    for bi, h in enumerate(seq):
                        tp = h // 2
                        if h % 2 == 1 and tp + 1 < CT:
                            filler.append(jit_chunks(tp + 1))
                            filler.append(jit_chunks(CT + tp + 1))
                        if h % 2 == 1 and (tp not in qkT or CT + tp not in qkT):
                            # this pair's q/k must be fully emitted first
                            while tp not in qkT or CT + tp not in qkT:
                                pop_filler(1)
                        if bi == 6:
                            load_wproj()
                        if bi == 8:
                            filler.append(wpt_chunks())
                        pvs = [(), (), (1,), (0,), (3,), (2,), (5,), (4,),
                               (7,), (6,), (9,), (10,)]
                        emit_head(h, qkT[tp], qkT[CT + tp], pvs[bi])

                    # trailing PVs: h8 (ready immediately, on the psJ bank)
                    # and h11 (gated on the final block's exps)
                    psv_alloc(8, "J")
                    for nt in range(NT):
                        emit_pv_nt(8, nt)
                        emit_norm_nt(8, nt)
                    del psv[8]
                    del ptiles[8]
                    psv_alloc(11, "V")

                    # attn-out -> c-major PE transposes pipelined with the
                    # projection: transpose(nt+1) hides proj(nt)'s wait on the
                    # attnT copy. (block (c, nt) of attnT at col c*N + nt*128)
                    def emit_trans(nt):
                        psg = psJ.tile([128, C], BF16, name="psgA", tag="J")
                        for c in range(CT):
                            nc.tensor.matmul(psg[:, c * 128:(c + 1) * 128],
                                             awide[nt][:, c * 128:(c + 1) * 128],
                                             ident_b[:], is_transpose=True,
                                             skip_group_check=True)
                        cpy(nt, attnT[:].rearrange("p (t nn) -> p t nn", t=CT)[
                            :, :, nt * 128:nt * 128 + 128], psg[:].rearrange(
                            "p (t nn) -> p t nn", t=CT))

                    def emit_proj(nt):
                        ps = psA.tile([128, C], F32, name="psC2", tag="A")
                        for hp in range(H // 2):
                            for f0, fw in ((0, 512), (512, 256)):
                                nc.tensor.matmul(
                                    ps[:, f0:f0 + fw],
                                    attnT[:, hp * N + nt * 128: hp * N + nt * 128 + 128],
                                    wpT[hp][:, f0:f0 + fw],
                                    start=(hp == 0), stop=(hp == H // 2 - 1))
                        yt = wp.tile([128, C], F32, name="yt", tag="yt")
                        nc.vector.tensor_tensor(yt[:], ps[:], b_bc[:], AluOpType.add)
                        nc.sync.dma_start(out=out_e[nt * 128:(nt + 1) * 128, :], in_=yt[:])

                    for nt in range(NT):
                        emit_pv_nt(11, nt)
                        emit_norm_nt(11, nt)
                    del psv[11]
                    del ptiles[11]
                    emit_trans(0)
                    for nt in range(1, NT):
                        emit_trans(nt)
                        emit_proj(nt - 1)
                    emit_proj(NT - 1)

    return nc


_NC = None


def _get_nc():
    global _NC
    if _NC is None:
        _NC = build_program()
    return _NC


def run(in_maps, trace=False, **kw):
    from concourse.bass_utils import run_bass_kernel_spmd
    return run_bass_kernel_spmd(_get_nc(), in_maps, core_ids=list(range(B)),
                                trace=trace, **kw)


def kernel(x, policy, w_qkv, w_proj, b_proj):
    x = np.ascontiguousarray(np.asarray(x, dtype=np.float32))
    policy = np.ascontiguousarray(np.asarray(policy, dtype=np.float32))
    w_qkv = np.ascontiguousarray(np.asarray(w_qkv, dtype=np.float32))
    w_proj = np.ascontiguousarray(np.asarray(w_proj, dtype=np.float32))
    b_proj = np.ascontiguousarray(np.asarray(b_proj, dtype=np.float32))
    in_maps = [
        {"x": x[i], "policy": policy[i], "w_qkv": w_qkv,
         "w_proj": w_proj, "b_proj": b_proj}
        for i in range(B)
    ]
    try:
        res = run(in_maps)
    except Exception:
        # one observed transient NRT_EXEC_UNIT_UNRECOVERABLE wedge in ~40
        # invocations this session; a retry is free insurance
        res = run(in_maps)
    return np.stack([res.results[i]["out"] for i in range(B)], axis=0)


if __name__ == "__main__":
    rng = np.random.default_rng(0)
    x = rng.standard_normal((B, N, C), dtype=np.float32)
    policy = (rng.random((B, N, 1)) > 0.3).astype(np.float32)
    w_qkv = rng.standard_normal((3 * C, C), dtype=np.float32) * C ** -0.5
    w_proj = rng.standard_normal((C, C), dtype=np.float32) * C ** -0.5
    b_proj = np.zeros((C,), dtype=np.float32)
    y = kernel(x=x, policy=policy, w_qkv=w_qkv, w_proj=w_proj, b_proj=b_proj)
    print("out", y.shape, y.dtype, np.abs(y).mean())



# revision 46
# speedup vs baseline: 1.0795x; 1.0074x over previous
"""Sparse (policy-masked) attention on 8 TRN2 NeuronCores.

Data-parallel over batch (B=8 -> one batch element per core, weights
replicated, no collectives). Per core:

  Prologue: x and the pair-0 q/k weight rows load first; x is PE-transposed
  (f32r), pair-0 Q^T/K^T are projected immediately (q on the JIT psum bank,
  k on the score bank) so the exp stream starts ~20us in. W_v transposes and
  the V projection run as filler chunks inside the first two score blocks,
  with V accumulating in the then-idle PV psum banks.

  Attention: heads are processed in pair order [1,0,3,2,...,10,11]. Per
  key-tile iteration: S^T = K Q^T (bf16, key-major so the policy mask is a
  per-partition ACT bias), a bf16 identity matmul restores the always-keep
  diagonal (+2^13 pre-scale cancels the -1024 mask bias exactly), then
  exp -> bf16 P^T tiles. PV for a two-block-lagged head interleaves per
  iteration in query-major form: out[128 queries, 65] = P^T-block
  (stationary) @ [V | 1] (moving) -- all 128 output partitions used, the
  denominator rides along as column 64, and each query-tile normalizes on
  DVE (reciprocal + per-partition scale) the moment its column finishes.
  The next pair's Q^T/K^T projection and, late in the run, the W_proj
  transposes are drip-fed between iterations as filler chunks.

  Tail: PV for heads 8 and 11, then attn-out is PE-transposed (bf16) to
  c-major and the output projection (bf16 x bf16) pipelines per n-tile
  with the transposes; bias add on DVE, stores on the sync queue.

Timing source: HW-calibrated cost-model timeline sim (no NTFF profiling
under this axon client). ~179.8 us/core vs 216.7 us for the v1 baseline;
rel err vs fp64 reference 5.5e-3 on hardware (bf16 operand rounding).
"""

import numpy as np

import concourse.bass as bass
import concourse.mybir as mybir
import concourse.tile as tile_mod
from concourse.alu_op_type import AluOpType
from concourse.masks import make_identity
from concourse.tile import TileContext


class TC(TileContext):
    """TileContext emitting at most one sync-wait per instruction.

    The pinned walrus rejects any instruction with >1 sem waits
    ("Too many sync wait commands", setupSyncWait), so excess waits are
    hoisted onto single-wait NoOps on the same engine right before the
    instruction, and the final drain is emitted as a drain chain.
    """

    _ww_counter = 0

    def _commit_instruction(self, inst, lazy_reg_writes: bool = True):
        si = getattr(inst, "sync_info", None)
        if si is not None and si.on_wait is not None and len(si.on_wait) > 1:
            waits = list(si.on_wait)
            for w in waits[:-1]:
                TC._ww_counter += 1
                nop = mybir.InstNoOp(
                    name=f"{inst.name}-ww{TC._ww_counter}",
                    engine=inst.engine,
                    sync_info=mybir.SyncInfo(on_wait=[w], on_update=[]),
                    bass_nofuse=True,
                )
                super()._commit_instruction(nop, lazy_reg_writes)
            inst.sync_info = mybir.SyncInfo(
                on_wait=waits[-1:], on_update=list(si.on_update))
        return super()._commit_instruction(inst, lazy_reg_writes)

    def _drain_and_barrier(self, tick_clock, wait_clock):
        drain_inst = self.nc.sync.drain()
        wait_clock.add_sem_waits(
            drain_inst.ins, tile_mod.ScopedClock({None: tick_clock.global_clock})
        )
        waits = list(drain_inst.ins.sync_info.on_wait)
        if len(waits) > 1:
            drain_inst.ins.sync_info = mybir.SyncInfo(on_wait=waits[:1], on_update=[])
            for w in waits[1:]:
                d2 = self.nc.sync.drain()
                d2.ins.sync_info = mybir.SyncInfo(on_wait=[w], on_update=[])
        self.nc.all_engine_barrier()
        assert self.sems is not None
        popped = self.nc._tile_sem_poison_stack.pop()
        assert popped is self._sem_poison
        self.nc.clear_and_free_semaphores(list(self.sems.allocated().values()))
        self.nc.all_engine_barrier()

N, C, H, HD = 1024, 768, 12, 64
B = 8
SCALE = HD ** -0.5
EPS = 1e-6
BIG = 1024.0          # mask bias magnitude (post-scale); exp(-1024) == 0
DVAL = 8192.0         # BIG / SCALE, exactly representable power of two
F32 = mybir.dt.float32
F32R = mybir.dt.float32r
BF16 = mybir.dt.bfloat16
AF = mybir.ActivationFunctionType
NT = N // 128      # 8 n-tiles
CT = C // 128      # 6 c-tiles
QKT = 2 * C // 128  # 12 c_out tiles for Q,K


def build_program():
    nc = bass.Bass()
    x_e = nc.declare_dram_parameter("x", [N, C], F32, isOutput=False)
    pol_e = nc.declare_dram_parameter("policy", [N, 1], F32, isOutput=False)
    wqkv_e = nc.declare_dram_parameter("w_qkv", [3 * C, C], F32, isOutput=False)
    wproj_e = nc.declare_dram_parameter("w_proj", [C, C], F32, isOutput=False)
    b_e = nc.declare_dram_parameter("b_proj", [C], F32, isOutput=False)
    out_e = nc.declare_dram_parameter("out", [N, C], F32, isOutput=True)

    with TC(nc) as tc:
        with tc.tile_pool(name="persist", bufs=1) as pp, \
             tc.tile_pool(name="psA", bufs=2, space="PSUM") as psA, \
             tc.tile_pool(name="psV1", bufs=1, space="PSUM") as psV1, \
             tc.tile_pool(name="psV2", bufs=1, space="PSUM") as psV2, \
             tc.tile_pool(name="psJ", bufs=1, space="PSUM") as psJ:

            # ---- constants ----
            ident = pp.tile([128, 128], F32, tag="ident")
            make_identity(nc, ident[:])
            ident_b = pp.tile([128, 128], BF16, tag="ident_b")
            nc.vector.tensor_copy(ident_b[:], ident[:])
            ident_r = pp.tile([128, 128], F32R, tag="ident_r")
            nc.vector.tensor_copy(ident_r[:], ident[:])
            pol_t = pp.tile([128, NT], F32, tag="pol")
            nc.sync.dma_start(out=pol_t[:], in_=pol_e.rearrange("(t p) o -> p (t o)", p=128))
            # Mask bias: (policy-1)*1024 -> 0 kept, -1024 dropped; exp(-1024)=0.
            logmask = pp.tile([128, NT], F32, tag="logmask")
            nc.vector.tensor_scalar(logmask[:], pol_t[:], -1.0, float(BIG),
                                    AluOpType.add, AluOpType.mult)
            # Diagonal unmask: add (1-policy[m])*2^13 to the raw score diagonal
            # so the ACT bias cancels exactly there (2^13 * SCALE == 1024).
            dpol = pp.tile([128, NT], F32, tag="dpol")
            nc.vector.tensor_scalar(dpol[:], pol_t[:], -1.0, -float(DVAL),
                                    AluOpType.add, AluOpType.mult)
            dmask = [pp.tile([128, 128], BF16, name=f"dmask{t}", tag=f"dmask{t}")
                     for t in range(NT)]
            for t in range(NT):
                nc.vector.tensor_scalar(dmask[t][:], ident[:], dpol[:, t:t + 1],
                                        None, AluOpType.mult)

            ones_f = pp.tile([128, H], F32, tag="ones_f")
            nc.gpsimd.memset(ones_f[:], 1.0)
            ones = pp.tile([128, H], BF16, tag="ones")
            nc.vector.tensor_copy(ones[:], ones_f[:])

            b_bc = pp.tile([128, C], F32, tag="b_bc")
            nc.sync.dma_start(
                out=b_bc[:],
                in_=b_e.rearrange("(o c) -> o c", o=1).to_broadcast([128, C]))

            # ---- persistent products ----
            vaug = [pp.tile([128, H * (HD + 1)], BF16, name=f"vaug{t}", tag=f"vaug{t}") for t in range(NT)]

            def cpy(i, out, in_):
                # alternate copies between DVE and ACT to halve the copy wall
                if i % 2 == 0:
                    nc.vector.tensor_copy(out, in_)
                else:
                    nc.scalar.copy(out, in_)

            def transpose(out, in_):
                nc.tensor.matmul(out, in_, ident_r[:], is_transpose=True,
                                 skip_group_check=True)

            # ---- phase 1: x transposes; V via JIT-transposed W_v ----
            with tc.tile_pool(name="loadL1", bufs=1) as l1p:
                xT = [l1p.tile([128, N], BF16, name=f"xT{c}", tag=f"xT{c}") for c in range(CT)]
                with tc.tile_pool(name="xrawp", bufs=1) as xrawp:
                    xraws = []
                    for nt in range(NT):
                        xr = xrawp.tile([128, C], F32R, name=f"xraw{nt}", tag=f"xraw{nt}")
                        nc.sync.dma_start(out=xr[:], in_=x_e[nt * 128:(nt + 1) * 128, :].bitcast(F32R))
                        xraws.append(xr)
                    for c in range(CT):
                        for half in range(2):
                            psg = psA.tile([128, N // 2], F32R, name="psg", tag="A")
                            for i in range(NT // 2):
                                nt = half * (NT // 2) + i
                                transpose(psg[:, i * 128:(i + 1) * 128],
                                          xraws[nt][:, c * 128:(c + 1) * 128])
                            nc.scalar.copy(
                                xT[c][:, half * (N // 2):(half + 1) * (N // 2)], psg[:])
                # pair-0 q/k rows load right behind x; wv after them
                qk0raw = {}
                for t0 in (0, CT):
                    wr0 = l1p.tile([128, C], F32R, name=f"qk0r{t0}", tag=f"qk0r{t0}")
                    nc.sync.dma_start(out=wr0[:], in_=wqkv_e[t0 * 128:(t0 + 1) * 128, :].bitcast(F32R))
                    qk0raw[t0] = wr0
                wvraws = []
                for i in range(CT):
                    rr = 2 * CT + i
                    wr = l1p.tile([128, C], F32R, name=f"wvraw{i}", tag=f"wvraw{i}")
                    nc.sync.dma_start(out=wr[:], in_=wqkv_e[rr * 128:(rr + 1) * 128, :].bitcast(F32R))
                    wvraws.append(wr)
                wvT = [l1p.tile([128, C], BF16, name=f"wvT{c}", tag=f"wvT{c}") for c in range(CT)]
                for nt in range(NT):
                    nc.vector.tensor_copy(
                        vaug[nt][:].rearrange("p (h e) -> p e h", e=HD + 1)[:, HD:HD + 1, :],
                        ones[:, 0:H].rearrange("p (o h) -> p o h", o=1))

                # ---- phase 3: attention with JIT qkv^T between heads ----
                with tc.tile_pool(name="loadL2", bufs=1) as l2p, \
                         tc.tile_pool(name="rawcyc2", bufs=1) as rawp2, \
                         tc.tile_pool(name="jitq", bufs=2) as jitq, \
                         tc.tile_pool(name="qkp", bufs=4) as qkp, \
                         tc.tile_pool(name="work", bufs=2) as wp, \
                         tc.tile_pool(name="ptp", bufs=24) as ptp, \
                         tc.tile_pool(name="epi", bufs=3) as ep:
                    awide = [l2p.tile([128, C], BF16, name=f"aw{t}", tag=f"aw{t}")
                             for t in range(NT)]
                    attnT = l2p.tile([128, CT * N], BF16, tag="attnT")

                    wpraws = []
                    wpT = [l2p.tile([128, C], BF16, name=f"wpT{p}", tag=f"wpT{p}")
                           for p in range(H // 2)]

                    def wpt_chunks():
                        for hp in range(H // 2):
                            def tchunk(hp=hp):
                                psg = psJ.tile([128, C], F32R, name="psg3", tag="J")
                                for rr in range(CT):
                                    transpose(psg[:, rr * 128:(rr + 1) * 128],
                                              wpraws[rr][:, hp * 128:(hp + 1) * 128])
                                cpy(hp, wpT[hp][:], psg[:])
                            yield tchunk

                    def load_wproj():
                        # deferred prefetch: issued mid-attention so it never
                        # competes with the x/w_qkv loads for DMA bandwidth
                        for rr in range(CT):
                            wpraw = rawp2.tile([128, C], F32R, name=f"wpraw{rr}", tag=f"wpraw{rr}")
                            nc.sync.dma_start(out=wpraw[:], in_=wproj_e[rr * 128:(rr + 1) * 128, :].bitcast(F32R))
                            wpraws.append(wpraw)

                    qkT = {}

                    def jit_chunks(t, pool=None):
                        """Chunked Q^T/K^T projection for w_qkv row-tile t."""
                        pool = pool or psJ
                        tag = "J" if pool is psJ else "A"
                        if t in qk0raw:
                            wqr = qk0raw[t]
                        else:
                            wqr = jitq.tile([128, C], F32R, name="wqr", tag="wqr")
                            nc.sync.dma_start(out=wqr[:], in_=wqkv_e[t * 128:(t + 1) * 128, :].bitcast(F32R))
                        psg = pool.tile([128, C], F32R, name="psgq", tag=tag)
                        def tchunk():
                            for c in range(CT):
                                transpose(psg[:, c * 128:(c + 1) * 128],
                                          wqr[:, c * 128:(c + 1) * 128])
                        yield tchunk
                        wqTt = jitq.tile([128, C], BF16, name="wqTt", tag="wqTt")
                        yield lambda: nc.vector.tensor_copy(wqTt[:], psg[:])
                        psq = pool.tile([128, N], F32, name="psJN", tag=tag)
                        for c0 in range(0, CT, 2):
                            for j in range(2):
                                def mm(c0=c0, j=j):
                                    for c in (c0, c0 + 1):
                                        nc.tensor.matmul(
                                            psq[:, j * 512:(j + 1) * 512],
                                            wqTt[:, c * 128:(c + 1) * 128],
                                            xT[c][:, j * 512:(j + 1) * 512],
                                            start=(c == 0), stop=(c == CT - 1),
                                            skip_group_check=True)
                                yield mm
                        def fin():
                            qo = qkp.tile([128, N], BF16, name="qkvTt", tag="qk")
                            nc.vector.tensor_copy(qo[:], psq[:])
                            qkT[t] = qo
                        yield fin

                    def wvt_chunks():
                        """W_v^T via PE transposes (chunked)."""
                        for c in range(CT):
                            def tchunk(c=c):
                                psg = psA.tile([128, C], F32R, name="psg2", tag="A")
                                for i in range(CT):
                                    transpose(psg[:, i * 128:(i + 1) * 128],
                                              wvraws[i][:, c * 128:(c + 1) * 128])
                                cpy(c, wvT[c][:], psg[:])
                            yield tchunk

                    def v_chunks(nt):
                        """V projection for n-tile nt into the (idle until the
                        first PV) psV banks; finishes with the bf16 vaug copy."""
                        a = psV1.tile([128, 512], F32, name=f"vA{nt}", tag="V1")
                        bt = psV2.tile([128, 512], F32, name=f"vB{nt}", tag="V2")
                        for c in range(CT):
                            def mm(c=c):
                                nc.tensor.matmul(
                                    a[:, 0:512],
                                    xT[c][:, nt * 128:(nt + 1) * 128],
                                    wvT[c][:, 0:512],
                                    start=(c == 0), stop=(c == CT - 1),
                                    skip_group_check=True)
                                nc.tensor.matmul(
                                    bt[:, 0:256],
                                    xT[c][:, nt * 128:(nt + 1) * 128],
                                    wvT[c][:, 512:768],
                                    start=(c == 0), stop=(c == CT - 1),
                                    skip_group_check=True)
                            yield mm
                        def fin():
                            nc.vector.tensor_copy(
                                vaug[nt][:].rearrange("p (h e) -> p h e", h=H)[:, 0:8, 0:HD],
                                a[:, 0:512].rearrange("p (h e) -> p h e", h=8))
                            nc.vector.tensor_copy(
                                vaug[nt][:].rearrange("p (h e) -> p h e", h=H)[:, 8:12, 0:HD],
                                bt[:, 0:256].rearrange("p (h e) -> p h e", h=4))
                        yield fin

                    filler = []

                    def pop_filler(budget):
                        done = 0
                        while filler and done < budget:
                            try:
                                next(filler[0])()
                                done += 1
                            except StopIteration:
                                filler.pop(0)

                    ptiles = {}   # h -> [8 bf16 P^T tiles]
                    psv = {}      # h -> (psum nt 0-3, psum nt 4-7)

                    def psv_alloc(h, pool="V"):
                        if pool == "V":
                            psv[h] = (
                                psV1.tile([128, 512], F32, name=f"pva{h}", tag="V1"),
                                psV2.tile([128, 512], F32, name=f"pvb{h}", tag="V2"))
                        else:
                            # the JIT psum bank is free late in the run; host a
                            # second concurrent PV stream there
                            tile = psJ.tile([128, N], F32, name=f"pvj{h}", tag="J")
                            psv[h] = (tile[:, 0:512], tile[:, 512:1024])

                    def emit_head(h, qt, kt, pv_hs):
                        """Scores+exp for head h; PV matmuls for the lagged
                        heads in pv_hs interleaved per key-tile, each
                        normalized per query-tile as its column finishes."""
                        rb = (h % 2) * 64
                        for i, pv_h in enumerate(pv_hs):
                            psv_alloc(pv_h, "V" if i == 0 else "J")
                        for mt in range(NT):
                            ps = psA.tile([128, N], F32, name="psN", tag="A")
                            # emit the diag-fix half first so its group closes
                            # before the second 512-wide matmul: the exp then
                            # waits only on the last big matmul
                            jm = mt // 4
                            nc.tensor.matmul(
                                ps[:, jm * 512:(jm + 1) * 512],
                                kt[rb:rb + HD, mt * 128:(mt + 1) * 128],
                                qt[rb:rb + HD, jm * 512:(jm + 1) * 512],
                                start=True, stop=False,
                                skip_group_check=True)
                            nc.tensor.matmul(
                                ps[:, mt * 128:(mt + 1) * 128],
                                ident_b[:], dmask[mt][:],
                                start=False, stop=True, skip_group_check=True)
                            jo = 1 - jm
                            nc.tensor.matmul(
                                ps[:, jo * 512:(jo + 1) * 512],
                                kt[rb:rb + HD, mt * 128:(mt + 1) * 128],
                                qt[rb:rb + HD, jo * 512:(jo + 1) * 512],
                                start=True, stop=True,
                                skip_group_check=True)
                            ptile = ptp.tile([128, N], BF16, name=f"pt{h}_{mt}", tag="pt")
                            nc.scalar.activation(ptile[:], ps[:], AF.Exp,
                                                 bias=logmask[:, mt:mt + 1], scale=SCALE)
                            ptiles.setdefault(h, []).append(ptile)
                            pop_filler(2 if pv_hs else 8)
                            for pv_h in pv_hs:
                                emit_pv_nt(pv_h, mt)
                                emit_norm_nt(pv_h, mt)
                        for pv_h in pv_hs:
                            del psv[pv_h]
                            del ptiles[pv_h]

                    def emit_pv_nt(h, nt):
                        # query-major PV for one query tile: stationary P^T
                        # block [128 keys, 128 queries], moving [V | 1]
                        # [128 keys, 65] -> psum [128 queries, 65] with the
                        # denominator in col 64. The 8 key-tile matmuls are
                        # contiguous: one PSUM bank allows only one open
                        # accumulation group at a time.
                        a, bt = psv[h]
                        tgt = a if nt < 4 else bt
                        off = (nt % 4) * 128
                        for mt in range(NT):
                            nc.tensor.matmul(
                                tgt[:, off:off + HD + 1],
                                ptiles[h][mt][:, nt * 128:(nt + 1) * 128],
                                vaug[mt][:, h * (HD + 1):(h + 1) * (HD + 1)],
                                start=(mt == 0), stop=(mt == NT - 1),
                                skip_group_check=True)

                    dens = {}

                    def emit_norm_nt(h, nt):
                        # normalize query-tile nt of head h right after its PV
                        # column finishes, so the next block's PV stream never
                        # waits on a batched norm chain
                        a, bt = psv[h]
                        t = a if nt < 4 else bt
                        off = (nt % 4) * 128
                        if nt == 0:
                            dens[h] = ep.tile([128, NT], F32, name=f"den{h}", tag="den")
                        den = dens[h]
                        nc.vector.tensor_scalar_add(
                            den[:, nt:nt + 1], t[:, off + HD:off + HD + 1], EPS)
                        nc.vector.reciprocal(den[:, nt:nt + 1], den[:, nt:nt + 1])
                        nc.vector.tensor_scalar(
                            awide[nt][:, h * HD:(h + 1) * HD],
                            t[:, off:off + HD], den[:, nt:nt + 1], None,
                            AluOpType.mult)

                    # pair-0 q/k projected up-front (before V, right after
                    # xT), round-robin so the q and k chains overlap across
                    # their two psum pools
                    g0, g1 = jit_chunks(0), jit_chunks(CT, pool=psA)
                    d0 = d1 = False
                    while not (d0 and d1):
                        if not d0:
                            try:
                                next(g0)()
                            except StopIteration:
                                d0 = True
                        if not d1:
                            try:
                                next(g1)()
                            except StopIteration:
                                d1 = True
                    # wvT + all of V run as filler inside the first two score
                    # blocks (heads 1 and 0); PV lags its head by two blocks.
                    filler.append(wvt_chunks())
                    for nt in range(NT):
                        filler.append(v_chunks(nt))

                    seq = []
                    for tp in range(CT):
                        seq += [2 * tp + 1, 2 * tp]
                    seq[10], seq[11] = 10, 11   # last pair even-first: PV(10)
                    # then rides block 11 and only PV(11) trails the loop
                    for bi, h in enumerate(seq):
                        tp = h // 2
                        if h % 2 == 1 and tp + 1 < CT:
                            filler.append(jit_chunks(tp + 1))
                            filler.append(jit_chunks(CT + tp + 1))
                        if h % 2 == 1 and (tp not in qkT or CT + tp not in qkT):
                            # this pair's q/k must be fully emitted first
                            while tp not in qkT or CT + tp not in qkT:
                                pop_filler(1)
                        if bi == 6:
                            load_wproj()
                        if bi == 8:
                            filler.append(wpt_chunks())
                        pvs = [(), (), (1,), (0,), (3,), (2,), (5,), (4,),
                               (7,), (6,), (9,), (10,)]
                        emit_head(h, qkT[tp], qkT[CT + tp], pvs[bi])

                    # trailing PVs: h8 (ready immediately, on the psJ bank)
                    # and h11 (gated on the final block's exps)
                    psv_alloc(8, "J")
                    for nt in range(NT):
                        emit_pv_nt(8, nt)
                        emit_norm_nt(8, nt)
                    del psv[8]
                    del ptiles[8]
                    psv_alloc(11, "V")

                    # attn-out -> c-major PE transposes pipelined with the
                    # projection: transpose(nt+1) hides proj(nt)'s wait on the
                    # attnT copy. (block (c, nt) of attnT at col c*N + nt*128)
                    def emit_trans(nt):
                        # alternate psum pools so transpose nt+1 never waits
                        # on transpose nt's PSUM->SBUF copy
                        if nt % 2 == 0:
                            psg = psJ.tile([128, C], BF16, name="psgA", tag="J")
                        else:
                            psg = psV1.tile([128, C], BF16, name="psgB", tag="V1")
                        for c in range(CT):
                            nc.tensor.matmul(psg[:, c * 128:(c + 1) * 128],
                                             awide[nt][:, c * 128:(c + 1) * 128],
                                             ident_b[:], is_transpose=True,
                                             skip_group_check=True)
                        cpy(nt, attnT[:].rearrange("p (t nn) -> p t nn", t=CT)[
                            :, :, nt * 128:nt * 128 + 128], psg[:].rearrange(
                            "p (t nn) -> p t nn", t=CT))

                    def emit_proj(nt):
                        ps = psA.tile([128, C], F32, name="psC2", tag="A")
                        for hp in range(H // 2):
                            for f0, fw in ((0, 512), (512, 256)):
                                nc.tensor.matmul(
                                    ps[:, f0:f0 + fw],
                                    attnT[:, hp * N + nt * 128: hp * N + nt * 128 + 128],
                                    wpT[hp][:, f0:f0 + fw],
                                    start=(hp == 0), stop=(hp == H // 2 - 1))
                        yt = wp.tile([128, C], F32, name="yt", tag="yt")
                        nc.vector.tensor_tensor(yt[:], ps[:], b_bc[:], AluOpType.add)
                        nc.sync.dma_start(out=out_e[nt * 128:(nt + 1) * 128, :], in_=yt[:])

                    for nt in range(NT):
                        emit_pv_nt(11, nt)
                        emit_norm_nt(11, nt)
                    del psv[11]
                    del ptiles[11]
                    emit_trans(0)
                    for nt in range(1, NT):
                        emit_trans(nt)
                        emit_proj(nt - 1)
                    emit_proj(NT - 1)

    return nc


_NC = None


def _get_nc():
    global _NC
    if _NC is None:
        _NC = build_program()
    return _NC


def run(in_maps, trace=False, **kw):
    from concourse.bass_utils import run_bass_kernel_spmd
    return run_bass_kernel_spmd(_get_nc(), in_maps, core_ids=list(range(B)),
                                trace=trace, **kw)


def kernel(x, policy, w_qkv, w_proj, b_proj):
    x = np.ascontiguousarray(np.asarray(x, dtype=np.float32))
    policy = np.ascontiguousarray(np.asarray(policy, dtype=np.float32))
    w_qkv = np.ascontiguousarray(np.asarray(w_qkv, dtype=np.float32))
    w_proj = np.ascontiguousarray(np.asarray(w_proj, dtype=np.float32))
    b_proj = np.ascontiguousarray(np.asarray(b_proj, dtype=np.float32))
    in_maps = [
        {"x": x[i], "policy": policy[i], "w_qkv": w_qkv,
         "w_proj": w_proj, "b_proj": b_proj}
        for i in range(B)
    ]
    try:
        res = run(in_maps)
    except Exception:
        # one observed transient NRT_EXEC_UNIT_UNRECOVERABLE wedge in ~40
        # invocations this session; a retry is free insurance
        res = run(in_maps)
    return np.stack([res.results[i]["out"] for i in range(B)], axis=0)


if __name__ == "__main__":
    rng = np.random.default_rng(0)
    x = rng.standard_normal((B, N, C), dtype=np.float32)
    policy = (rng.random((B, N, 1)) > 0.3).astype(np.float32)
    w_qkv = rng.standard_normal((3 * C, C), dtype=np.float32) * C ** -0.5
    w_proj = rng.standard_normal((C, C), dtype=np.float32) * C ** -0.5
    b_proj = np.zeros((C,), dtype=np.float32)
    y = kernel(x=x, policy=policy, w_qkv=w_qkv, w_proj=w_proj, b_proj=b_proj)
    print("out", y.shape, y.dtype, np.abs(y).mean())

